# revision 1
# baseline (speedup 1.0000x reference)
"""GCN layer (gather -> x@W -> normalized scatter-add -> bias -> PReLU) on 8 trn2 cores.

Strategy (node sharding):
  - 100000 nodes padded to 102400 = 8 * 12800; core c owns nodes [c*12800, (c+1)*12800).
  - Phase 1: each core computes hs = dinv * (x_own @ W) for its nodes (fp32 on PE, x tiles
    transposed via PE), written as bf16 in 4 quarter tensors (3200 rows each).
  - Phase 2: 4 AllGathers (one per quarter) -> 4 shared tables [25600, 128] bf16; each
    pipelines behind the quarter's phase-1 writes and ahead of phase-3 consumers.
  - Phase 3: edges sorted by destination; destinations processed in 128-node windows
    (PSUM [128 dst, 128 feat], 8 windows in flight); per 128-edge block a one-hot
    S [edge, dst] is built on DVE (iota == reldst) and PE accumulates psum += S^T @ G,
    where G = dma_gather'ed hs rows (int16 idx into the 25600-row quarter table).
    Self-loops are one identity matmul per window on contiguous hs rows.
    Epilogue: out = prelu(dinv_dst * psum + b) with per-feature alpha.
"""
import sys
sys.path.insert(0, '/opt/trn_rl_repo')

import numpy as np
import ml_dtypes

N = 100000
NCORES = 8
SH = 12800                 # nodes per core
NP = NCORES * SH           # 102400 padded nodes
H = 128                    # output features
KIN = 256                  # input features
WIN = 128                  # dst window size
NW = SH // WIN             # 100 windows per core
WG = 8                     # windows per PSUM group
NG = (NW + WG - 1) // WG   # 13 groups (last has 4)
NQ = 4                     # source quarters
QSH = SH // NQ             # 3200 rows of own shard per quarter
QT = QSH // WIN            # 25 tiles per quarter
TAB = NCORES * QSH         # 25600 rows per gather table (int16-safe)
XB = 5                     # phase-1 tiles per DMA batch (25 tiles/quarter = 5 batches)

bf16 = ml_dtypes.bfloat16


def _preprocess(edge_index):
    src = np.asarray(edge_index[0]).astype(np.int64)
    dst = np.asarray(edge_index[1]).astype(np.int64)
    E = src.shape[0]

    deg = (np.bincount(dst, minlength=N) + 1).astype(np.float32)
    dinv = (1.0 / np.sqrt(deg)).astype(np.float32)
    dinv_np = np.ones(NP, np.float32)
    dinv_np[:N] = dinv

    core = dst // SH
    w_in_core = (dst % SH) // WIN            # 0..NW-1
    g = w_in_core // WG
    wi = w_in_core % WG
    # source quarter + row in its gather table
    s_core = src // SH
    s_li = src % SH
    q = s_li // QSH
    tab_row = s_core * QSH + (s_li % QSH)    # < TAB

    key = ((core * NG + g) * NQ + q) * WG + wi
    nbins_pc = NG * NQ * WG
    order = np.argsort(key, kind='stable')
    o_tab = tab_row[order]
    o_dst = dst[order]
    o_key = key[order]

    cnt_all = np.bincount(key, minlength=NCORES * nbins_pc)
    bin_start = np.concatenate([[0], np.cumsum(cnt_all)])[:-1]
    rank = np.arange(E, dtype=np.int64) - bin_start[o_key]

    cnt = cnt_all.reshape(NCORES, NG, NQ, WG)
    nblk = np.ceil(cnt.max(axis=0) / WIN).astype(np.int64)   # [NG, NQ, WG] common
    pad_sizes = (nblk * WIN).reshape(-1)
    offs = np.concatenate([[0], np.cumsum(pad_sizes)])
    TOT = int(offs[-1])
    NBLK = TOT // WIN

    bin_in_core = o_key % nbins_pc
    pos = offs[bin_in_core] + rank
    win_base = core * SH + w_in_core * WIN
    o_win_base = win_base[order]
    o_core = core[order]

    per_core = []
    for c in range(NCORES):
        m = o_core == c
        idxq = np.zeros(TOT, np.int16)
        rels = np.full(TOT, -1.0, np.float32)
        p_c = pos[m]
        idxq[p_c] = o_tab[m].astype(np.int16)
        rels[p_c] = (o_dst[m] - o_win_base[m]).astype(np.float32)
        idx16 = np.tile(np.ascontiguousarray(idxq.reshape(TOT // 16, 16).T), (8, 1))
        relm = np.ascontiguousarray(rels.reshape(NBLK, WIN).T)   # [128, NBLK]
        dinv_own = np.ascontiguousarray(
            dinv_np[c * SH:(c + 1) * SH].reshape(NW, WIN).T)     # [128, NW]
        per_core.append(dict(idx16=idx16, reldst=relm, dinv=dinv_own))

    calls = []          # (g, q, off_idx, nidx, [(Bcol, w), ...])
    Bcol = 0
    last_block_of_win = {}
    for gg in range(NG):
        for qq in range(NQ):
            blocks = []
            off_idx = None
            for wii in range(WG):
                nb = int(nblk[gg, qq, wii])
                if nb == 0:
                    continue
                w = gg * WG + wii
                if w >= NW:
                    continue
                bin_i = (gg * NQ + qq) * WG + wii
                if off_idx is None:
                    off_idx = int(offs[bin_i])
                for k in range(nb):
                    blocks.append((Bcol, w))
                    last_block_of_win[w] = Bcol
                    Bcol += 1
            if blocks:
                calls.append((gg, qq, off_idx, len(blocks) * WIN, blocks))
    sched = dict(calls=calls, last_block=last_block_of_win, NBLK=NBLK, TOT=TOT)
    return sched, per_core, dinv_np


def _build(sched):
    import os
    from concourse import bass, bacc, tile, mybir
    from concourse.masks import make_identity

    nc = bacc.Bacc("TRN2", target_bir_lowering=False, debug=False,
                   enable_asserts=True, num_devices=NCORES)

    x_d = nc.dram_tensor("x_own", [SH, KIN], mybir.dt.float32, kind="ExternalInput")
    w_d = nc.dram_tensor("w_mat", [KIN, H], mybir.dt.float32, kind="ExternalInput")
    b_d = nc.dram_tensor("b_vec", [H], mybir.dt.float32, kind="ExternalInput")
    a_d = nc.dram_tensor("a_vec", [H], mybir.dt.float32, kind="ExternalInput")
    dinv_d = nc.dram_tensor("dinv_own", [128, NW], mybir.dt.float32, kind="ExternalInput")
    idx_d = nc.dram_tensor("idx16", [128, sched["TOT"] // 16], mybir.dt.int16, kind="ExternalInput")
    rel_d = nc.dram_tensor("reldst", [128, sched["NBLK"]], mybir.dt.float32, kind="ExternalInput")

    out_d = nc.dram_tensor("out_own", [SH, H], mybir.dt.float32, kind="ExternalOutput")

    hs_q = [nc.dram_tensor(f"hs_q{k}", [QSH, H], mybir.dt.bfloat16) for k in range(NQ)]
    hs_tab = [nc.dram_tensor(f"hs_tab{k}", [TAB, H], mybir.dt.bfloat16, addr_space="Shared")
              for k in range(NQ)]

    calls = sched["calls"]
    last_block = sched["last_block"]
    max_call_blk = max(len(cb[4]) for cb in calls)
    nblk_of_group = [sum(len(cb[4]) for cb in calls if cb[0] == gg) for gg in range(NG)]
    first_col_of_group = [min([cb[4][0][0] for cb in calls if cb[0] == gg] or [0])
                          for gg in range(NG)]

    with tile.TileContext(nc) as tc:
        with tc.tile_pool(name="consts", bufs=1) as cp, tc.tile_pool(name="sb", bufs=3) as sb:
            # ---------------- constants ----------------
            iota_i = cp.tile([128, 128], mybir.dt.int32)
            nc.gpsimd.iota(iota_i[:], pattern=[[1, 128]], base=0, channel_multiplier=0)
            iota_f = cp.tile([128, 128], mybir.dt.float32)
            nc.vector.tensor_copy(iota_f[:], iota_i[:])
            iota_b = cp.tile([128, 128], mybir.dt.bfloat16)
            nc.vector.tensor_copy(iota_b[:], iota_i[:])

            ident_f = cp.tile([128, 128], mybir.dt.float32)
            make_identity(nc, ident_f[:])
            ident_b = cp.tile([128, 128], mybir.dt.bfloat16)
            nc.vector.tensor_copy(ident_b[:], ident_f[:])

            w0 = cp.tile([128, H], mybir.dt.float32)
            w1 = cp.tile([128, H], mybir.dt.float32)
            nc.sync.dma_start(w0[:], w_d[0:128, :])
            nc.sync.dma_start(w1[:], w_d[128:256, :])

            dinv_sb = cp.tile([128, NW], mybir.dt.float32)
            nc.sync.dma_start(dinv_sb[:], dinv_d[:, :])

            ones1 = cp.tile([1, H], mybir.dt.float32)
            nc.vector.memset(ones1[:], 1.0)
            bvec = cp.tile([1, H], mybir.dt.float32)
            nc.sync.dma_start(bvec[:], b_d[None, :])
            avec = cp.tile([1, H], mybir.dt.float32)
            nc.sync.dma_start(avec[:], a_d[None, :])

            b128 = cp.tile([128, H], mybir.dt.float32)
            a128 = cp.tile([128, H], mybir.dt.float32)
            hs_sb = cp.tile([128, NW * H], mybir.dt.bfloat16)   # own hs, SBUF-resident

            with tc.tile_pool(name="psum1", bufs=1, space="PSUM") as pp1:
                bc_ps = pp1.tile([128, H], mybir.dt.float32, space="PSUM", tag="bc", bufs=1)
                nc.tensor.matmul(out=bc_ps[:], lhsT=ones1[:], rhs=bvec[:], start=True, stop=True)
                nc.vector.tensor_copy(b128[:], bc_ps[:])
                ac_ps = pp1.tile([128, H], mybir.dt.float32, space="PSUM", tag="bc", bufs=1)
                nc.tensor.matmul(out=ac_ps[:], lhsT=ones1[:], rhs=avec[:], start=True, stop=True)
                nc.vector.tensor_copy(a128[:], ac_ps[:])

                # ---------------- phase 1 (+ per-quarter AllGather) ----------------
                for bb in range(NW // XB):          # batches of XB tiles
                    qk = bb // (QT // XB)           # quarter of this batch
                    t0 = bb * XB
                    x_t = sb.tile([128, XB * KIN], mybir.dt.float32, tag="x_t", bufs=3)
                    nc.sync.dma_start(
                        x_t[:],
                        x_d[t0 * 128:(t0 + XB) * 128, :].rearrange(
                            "(t p) k -> p t k", p=128))
                    hs_b = hs_sb[:, t0 * H:(t0 + XB) * H]
                    for tt in range(XB):
                        i = t0 + tt
                        h_ps = pp1.tile([128, H], mybir.dt.float32, space="PSUM",
                                        tag="h_ps", bufs=3)
                        for kk in range(2):
                            xt_ps = pp1.tile([128, 128], mybir.dt.float32, space="PSUM",
                                             tag="xt_ps", bufs=4)
                            nc.tensor.transpose(
                                xt_ps[:], x_t[:, (tt * 2 + kk) * 128:(tt * 2 + kk + 1) * 128],
                                ident_f[:])
                            xt_sb = sb.tile([128, 128], mybir.dt.float32, tag="xt_sb", bufs=4)
                            nc.vector.tensor_copy(xt_sb[:], xt_ps[:])
                            nc.tensor.matmul(out=h_ps[:], lhsT=xt_sb[:],
                                             rhs=(w0 if kk == 0 else w1)[:],
                                             start=(kk == 0), stop=(kk == 1))
                        nc.scalar.activation(hs_b[:, tt * H:(tt + 1) * H], h_ps[:],
                                             mybir.ActivationFunctionType.Copy,
                                             scale=dinv_sb[:, i:i + 1])
                    r0 = t0 * 128 - qk * QSH
                    nc.sync.dma_start(
                        hs_q[qk][r0:r0 + XB * 128, :].rearrange("(t p) k -> p t k", p=128),
                        hs_b)
                    if (bb + 1) % (QT // XB) == 0:
                        if os.environ.get("K_FAKE_COLL"):
                            # timing probe only: dependency-equivalent local copy
                            nc.sync.dma_start(hs_tab[qk][0:QSH, :], hs_q[qk][:, :])
                        else:
                            nc.gpsimd.collective_compute(
                                "AllGather", mybir.AluOpType.bypass,
                                replica_groups=[list(range(NCORES))],
                                ins=[hs_q[qk].ap().opt()],
                                outs=[hs_tab[qk].ap().opt()],
                            )

            # ---------------- phase 3 ----------------
            with tc.tile_pool(name="psum3", bufs=WG, space="PSUM") as pp3:
                for gg in range(NG):
                    wlo = gg * WG
                    whi = min(wlo + WG, NW)
                    nwin = whi - wlo
                    pw = {}
                    for w in range(wlo, whi):
                        pwt = pp3.tile([128, H], mybir.dt.float32, space="PSUM",
                                       tag="pw", name=f"pw{w}", bufs=8)
                        pw[w] = pwt[:]
                        nc.tensor.matmul(out=pw[w], lhsT=ident_b[:],
                                         rhs=hs_sb[:, w * H:(w + 1) * H],
                                         start=True, stop=(w not in last_block))

                    if nblk_of_group[gg]:
                        rd_sb = sb.tile([128, max(nblk_of_group)], mybir.dt.float32,
                                        tag="rd", bufs=3)
                        c0 = first_col_of_group[gg]
                        nc.sync.dma_start(rd_sb[:, 0:nblk_of_group[gg]],
                                          rel_d[:, c0:c0 + nblk_of_group[gg]])

                    for (g_c, qq, off_idx, nidx, blocks) in calls:
                        if g_c != gg:
                            continue
                        idx_sb = sb.tile([128, max_call_blk * 8], mybir.dt.int16,
                                         tag="idx", bufs=4)
                        nc.sync.dma_start(idx_sb[:, 0:nidx // 16],
                                          idx_d[:, off_idx // 16: (off_idx + nidx) // 16])
                        g_t = sb.tile([128, max_call_blk, H], mybir.dt.bfloat16,
                                      tag="g_t", bufs=5)
                        nc.gpsimd.dma_gather(
                            g_t[:, 0:nidx // 128, :], hs_tab[qq][:, :],
                            idx_sb[:, 0:nidx // 16], nidx, nidx, H,
                            single_packet=False)
                        for (bcol, w) in blocks:
                            s_t = sb.tile([128, 128], mybir.dt.bfloat16, tag="s_t", bufs=8)
                            lc = bcol - first_col_of_group[gg]
                            nc.vector.tensor_scalar(
                                out=s_t[:], in0=iota_b[:],
                                scalar1=rd_sb[:, lc:lc + 1], scalar2=None,
                                op0=mybir.AluOpType.is_equal)
                            slot = (bcol - blocks[0][0])
                            nc.tensor.matmul(out=pw[w], lhsT=s_t[:], rhs=g_t[:, slot, :],
                                             start=False, stop=(last_block.get(w) == bcol))

                    # epilogue, batched output DMA per group
                    o_g = sb.tile([128, WG * H], mybir.dt.float32, tag="o_g", bufs=2)
                    for w in range(wlo, whi):
                        u = sb.tile([128, H], mybir.dt.float32, tag="u", bufs=4)
                        nc.scalar.activation(u[:], pw[w], mybir.ActivationFunctionType.Copy,
                                             scale=dinv_sb[:, w:w + 1])
                        u2 = sb.tile([128, H], mybir.dt.float32, tag="u2", bufs=4)
                        nc.vector.tensor_tensor(out=u2[:], in0=u[:], in1=b128[:],
                                                op=mybir.AluOpType.add)
                        r2 = sb.tile([128, H], mybir.dt.float32, tag="r2", bufs=3)
                        nc.scalar.activation(r2[:], u2[:], mybir.ActivationFunctionType.Relu,
                                             scale=-1.0)
                        m = sb.tile([128, H], mybir.dt.float32, tag="m", bufs=3)
                        nc.gpsimd.tensor_tensor(out=m[:], in0=r2[:], in1=a128[:],
                                                op=mybir.AluOpType.mult)
                        r1 = sb.tile([128, H], mybir.dt.float32, tag="r1", bufs=3)
                        nc.scalar.activation(r1[:], u2[:], mybir.ActivationFunctionType.Relu)
                        nc.vector.tensor_tensor(out=o_g[:, (w - wlo) * H:(w - wlo + 1) * H],
                                                in0=r1[:], in1=m[:],
                                                op=mybir.AluOpType.subtract)
                    nc.sync.dma_start(
                        out_d[wlo * 128:whi * 128, :].rearrange("(t p) k -> p t k", p=128),
                        o_g[:, 0:nwin * H])

    nc.compile()
    return nc


_LAST = {}


def kernel(x, edge_index, W, b, alpha):
    from concourse.bass_utils import run_bass_kernel_spmd

    x = np.asarray(x, dtype=np.float32)
    W = np.asarray(W, dtype=np.float32)
    b = np.asarray(b, dtype=np.float32)
    alpha = np.asarray(alpha, dtype=np.float32)

    sched, per_core, dinv_np = _preprocess(edge_index)
    nc = _build(sched)
    _LAST["nc"] = nc
    _LAST["sched"] = sched

    x_pad = np.zeros((NP, KIN), np.float32)
    x_pad[:N] = x

    in_maps = []
    for c in range(NCORES):
        in_maps.append({
            "x_own": np.ascontiguousarray(x_pad[c * SH:(c + 1) * SH]),
            "w_mat": W, "b_vec": b, "a_vec": alpha,
            "dinv_own": per_core[c]["dinv"],
            "idx16": per_core[c]["idx16"],
            "reldst": per_core[c]["reldst"],
        })

    res = run_bass_kernel_spmd(nc, in_maps, core_ids=list(range(NCORES)))
    out = np.concatenate([res.results[c]["out_own"] for c in range(NCORES)], axis=0)
    return np.ascontiguousarray(out[:N])



# revision 2
# speedup vs baseline: 1.3522x; 1.3522x over previous
"""GCN layer (gather -> x@W -> normalized scatter-add -> bias -> PReLU) on 8 trn2 cores.

Strategy (no collectives; x replicated, full hs table computed locally per core):
  - 100000 nodes padded to 102400 = 8 * 12800; core c owns dst nodes [c*12800, (c+1)*12800).
  - Host: x' = dinv[:,None] * x (folds the src-side norm), padded, cast bf16,
    transposed to K-major [256, 102400], columns permuted so that phase-1 matmul
    chunk (g, c) yields, at out partition p, node g*512 + p*4 + c. Replicated to
    all 8 cores (51.2MB DMA each; cheaper in the cost model than any collective).
  - Phase 1: per 512-node group g: 8 matmuls (k=256 split in 2) into PSUM
    [128, 4*128], one Activation copy to bf16 SBUF, one DMA to the hs table with
    1KB-contiguous elements (4 consecutive rows per partition: row = g*512+p*4+c).
    Table stored as 4 quarter tensors [25600, 128] bf16 so gather idx fit int16.
  - Phase 2: none (no collective, no cross-core traffic).
  - Phase 3: edges (self-loops appended as real edges on host) sorted by
    (dst window group, src quarter, window); per 128-edge block a one-hot
    S[e, d] = (iota == reldst[e]) is built (alternating DVE/Pool), and PE
    accumulates psum[d, f] += S^T @ G where G = dma_gather'ed hs rows.
    Epilogue: out = prelu(dinv_dst * psum + b); output written partition-major
    [128, NW*H] (4KB contiguous) and transposed back on host.
"""
import sys
sys.path.insert(0, '/opt/trn_rl_repo')

import numpy as np
import ml_dtypes

N = 100000
NCORES = 8
SH = 12800                 # dst nodes per core
NP = NCORES * SH           # 102400 padded nodes
H = 128                    # output features
KIN = 256                  # input features
WIN = 128                  # dst window size
NW = SH // WIN             # 100 windows per core
WG = 8                     # windows per PSUM group
NG = (NW + WG - 1) // WG   # 13 groups (last has 4)
NQ = 4                     # source quarters (int16 gather idx: 25600 < 32768)
QTAB = NP // NQ            # 25600 rows per quarter table
GRP = 512                  # phase-1 rows per PSUM group
NGRP = NP // GRP           # 200 phase-1 groups

bf16 = ml_dtypes.bfloat16


def _preprocess(edge_index):
    e_src = np.asarray(edge_index[0]).astype(np.int64)
    e_dst = np.asarray(edge_index[1]).astype(np.int64)

    deg = (np.bincount(e_dst, minlength=N) + 1).astype(np.float32)
    dinv = (1.0 / np.sqrt(deg)).astype(np.float32)
    dinv_np = np.ones(NP, np.float32)
    dinv_np[:N] = dinv

    # self-loops as real edges (incl. padding nodes; their output is discarded)
    loops = np.arange(NP, dtype=np.int64)
    src = np.concatenate([e_src, loops])
    dst = np.concatenate([e_dst, loops])
    E = src.shape[0]

    core = dst // SH
    w_in_core = (dst % SH) // WIN            # 0..NW-1
    g = w_in_core // WG
    wi = w_in_core % WG
    q = src // QTAB                          # source quarter (canonical table order)
    tab_row = src % QTAB                     # row within quarter table

    key = ((core * NG + g) * NQ + q) * WG + wi
    nbins_pc = NG * NQ * WG
    order = np.argsort(key, kind='stable')
    o_tab = tab_row[order]
    o_dst = dst[order]
    o_key = key[order]

    cnt_all = np.bincount(key, minlength=NCORES * nbins_pc)
    bin_start = np.concatenate([[0], np.cumsum(cnt_all)])[:-1]
    rank = np.arange(E, dtype=np.int64) - bin_start[o_key]

    cnt = cnt_all.reshape(NCORES, NG, NQ, WG)
    nblk = np.ceil(cnt.max(axis=0) / WIN).astype(np.int64)   # [NG, NQ, WG] common
    pad_sizes = (nblk * WIN).reshape(-1)
    offs = np.concatenate([[0], np.cumsum(pad_sizes)])
    TOT = int(offs[-1])
    NBLK = TOT // WIN

    bin_in_core = o_key % nbins_pc
    pos = offs[bin_in_core] + rank
    win_base = core * SH + w_in_core * WIN
    o_win_base = win_base[order]
    o_core = core[order]

    per_core = []
    for c in range(NCORES):
        m = o_core == c
        idxq = np.zeros(TOT, np.int16)
        rels = np.full(TOT, -1.0, np.float32)
        p_c = pos[m]
        idxq[p_c] = o_tab[m].astype(np.int16)
        rels[p_c] = (o_dst[m] - o_win_base[m]).astype(np.float32)
        idx16 = np.tile(np.ascontiguousarray(idxq.reshape(TOT // 16, 16).T), (8, 1))
        relm = np.ascontiguousarray(rels.reshape(NBLK, WIN).T)   # [128, NBLK]
        dinv_own = np.ascontiguousarray(
            dinv_np[c * SH:(c + 1) * SH].reshape(NW, WIN).T)     # [128, NW]
        per_core.append(dict(idx16=idx16, reldst=relm, dinv=dinv_own))

    calls = []          # (g, q, off_idx, nidx, [(Bcol, w), ...])
    Bcol = 0
    first_block_of_win = {}
    last_block_of_win = {}
    for gg in range(NG):
        for qq in range(NQ):
            blocks = []
            off_idx = None
            for wii in range(WG):
                nb = int(nblk[gg, qq, wii])
                if nb == 0:
                    continue
                w = gg * WG + wii
                if w >= NW:
                    continue
                bin_i = (gg * NQ + qq) * WG + wii
                if off_idx is None:
                    off_idx = int(offs[bin_i])
                for k in range(nb):
                    blocks.append((Bcol, w))
                    first_block_of_win.setdefault(w, Bcol)
                    last_block_of_win[w] = Bcol
                    Bcol += 1
            if blocks:
                calls.append((gg, qq, off_idx, len(blocks) * WIN, blocks))
    sched = dict(calls=calls, first_block=first_block_of_win,
                 last_block=last_block_of_win, NBLK=NBLK, TOT=TOT)
    return sched, per_core, dinv_np


def _build(sched):
    from concourse import bass, bacc, tile, mybir

    nc = bacc.Bacc("TRN2", target_bir_lowering=False, debug=False,
                   enable_asserts=True, num_devices=NCORES)

    xt_d = nc.dram_tensor("xt_perm", [KIN, NP], mybir.dt.bfloat16, kind="ExternalInput")
    w_d = nc.dram_tensor("w_bf", [KIN, H], mybir.dt.bfloat16, kind="ExternalInput")
    b_d = nc.dram_tensor("b_vec", [H], mybir.dt.float32, kind="ExternalInput")
    a_d = nc.dram_tensor("a_vec", [H], mybir.dt.float32, kind="ExternalInput")
    dinv_d = nc.dram_tensor("dinv_own", [128, NW], mybir.dt.float32, kind="ExternalInput")
    idx_d = nc.dram_tensor("idx16", [128, sched["TOT"] // 16], mybir.dt.int16, kind="ExternalInput")
    rel_d = nc.dram_tensor("reldst", [128, sched["NBLK"]], mybir.dt.float32, kind="ExternalInput")

    # output in partition-major layout: out[d, w*H + f] = result[w*128 + d, f]
    out_d = nc.dram_tensor("out_pm", [128, NW * H], mybir.dt.float32, kind="ExternalOutput")

    hs_tab = [nc.dram_tensor(f"hs_tab{k}", [QTAB, H], mybir.dt.bfloat16) for k in range(NQ)]

    calls = sched["calls"]
    first_block = sched["first_block"]
    last_block = sched["last_block"]
    max_call_blk = max(len(cb[4]) for cb in calls)
    nblk_of_group = [sum(len(cb[4]) for cb in calls if cb[0] == gg) for gg in range(NG)]
    first_col_of_group = [min([cb[4][0][0] for cb in calls if cb[0] == gg] or [0])
                          for gg in range(NG)]

    with tile.TileContext(nc) as tc:
        with tc.tile_pool(name="consts", bufs=1) as cp, tc.tile_pool(name="sb", bufs=3) as sb:
            # ---------------- constants ----------------
            iota_i = cp.tile([128, 128], mybir.dt.int32)
            nc.gpsimd.iota(iota_i[:], pattern=[[1, 128]], base=0, channel_multiplier=0)
            iota_b = cp.tile([128, 128], mybir.dt.bfloat16)
            nc.vector.tensor_copy(iota_b[:], iota_i[:])

            w0 = cp.tile([128, H], mybir.dt.bfloat16)
            w1 = cp.tile([128, H], mybir.dt.bfloat16)
            nc.sync.dma_start(w0[:], w_d[0:128, :])
            nc.sync.dma_start(w1[:], w_d[128:256, :])

            dinv_sb = cp.tile([128, NW], mybir.dt.float32)
            nc.sync.dma_start(dinv_sb[:], dinv_d[:, :])

            ones1 = cp.tile([1, H], mybir.dt.float32)
            nc.vector.memset(ones1[:], 1.0)
            bvec = cp.tile([1, H], mybir.dt.float32)
            nc.sync.dma_start(bvec[:], b_d[None, :])
            avec = cp.tile([1, H], mybir.dt.float32)
            nc.sync.dma_start(avec[:], a_d[None, :])

            b128 = cp.tile([128, H], mybir.dt.float32)
            a128 = cp.tile([128, H], mybir.dt.float32)

            with tc.tile_pool(name="psum1", bufs=1, space="PSUM") as pp1:
                bc_ps = pp1.tile([128, H], mybir.dt.float32, space="PSUM", tag="bc", bufs=1)
                nc.tensor.matmul(out=bc_ps[:], lhsT=ones1[:], rhs=bvec[:], start=True, stop=True)
                nc.vector.tensor_copy(b128[:], bc_ps[:])
                ac_ps = pp1.tile([128, H], mybir.dt.float32, space="PSUM", tag="bc", bufs=1)
                nc.tensor.matmul(out=ac_ps[:], lhsT=ones1[:], rhs=avec[:], start=True, stop=True)
                nc.vector.tensor_copy(a128[:], ac_ps[:])

                # ---------------- phase 1: full hs table, local ----------------
                for g in range(NGRP):
                    x_t = sb.tile([128, 2, GRP], mybir.dt.bfloat16, tag="x_t", bufs=3)
                    nc.sync.dma_start(
                        x_t[:],
                        xt_d[:, g * GRP:(g + 1) * GRP].rearrange(
                            "(a p) c -> p a c", p=128))
                    ps = pp1.tile([128, 4 * H], mybir.dt.float32, space="PSUM",
                                  tag="h_ps", bufs=3)
                    for cc in range(4):
                        for a in range(2):
                            nc.tensor.matmul(
                                out=ps[:, cc * H:(cc + 1) * H],
                                lhsT=x_t[:, a, cc * 128:(cc + 1) * 128],
                                rhs=(w0 if a == 0 else w1)[:],
                                start=(a == 0), stop=(a == 1))
                    hb = sb.tile([128, 4 * H], mybir.dt.bfloat16, tag="hb", bufs=3)
                    nc.scalar.activation(hb[:], ps[:],
                                         mybir.ActivationFunctionType.Copy)
                    qk, r0 = divmod(g * GRP, QTAB)
                    nc.scalar.dma_start(
                        hs_tab[qk][r0:r0 + GRP, :].rearrange("(p c) k -> p (c k)", c=4),
                        hb[:])

            # ---------------- phase 3 ----------------
            with tc.tile_pool(name="psum3", bufs=WG, space="PSUM") as pp3:
                for gg in range(NG):
                    wlo = gg * WG
                    whi = min(wlo + WG, NW)
                    nwin = whi - wlo
                    pw = {}
                    for w in range(wlo, whi):
                        pwt = pp3.tile([128, H], mybir.dt.float32, space="PSUM",
                                       tag="pw", name=f"pw{w}", bufs=8)
                        pw[w] = pwt[:]

                    if nblk_of_group[gg]:
                        rd_sb = sb.tile([128, max(nblk_of_group)], mybir.dt.float32,
                                        tag="rd", bufs=3)
                        c0 = first_col_of_group[gg]
                        nc.sync.dma_start(rd_sb[:, 0:nblk_of_group[gg]],
                                          rel_d[:, c0:c0 + nblk_of_group[gg]])

                    for (g_c, qq, off_idx, nidx, blocks) in calls:
                        if g_c != gg:
                            continue
                        idx_sb = sb.tile([128, max_call_blk * 8], mybir.dt.int16,
                                         tag="idx", bufs=4)
                        nc.sync.dma_start(idx_sb[:, 0:nidx // 16],
                                          idx_d[:, off_idx // 16: (off_idx + nidx) // 16])
                        g_t = sb.tile([128, max_call_blk, H], mybir.dt.bfloat16,
                                      tag="g_t", bufs=5)
                        nc.gpsimd.dma_gather(
                            g_t[:, 0:nidx // 128, :], hs_tab[qq][:, :],
                            idx_sb[:, 0:nidx // 16], nidx, nidx, H,
                            single_packet=False)
                        for (bcol, w) in blocks:
                            s_t = sb.tile([128, 128], mybir.dt.bfloat16, tag="s_t", bufs=8)
                            lc = bcol - first_col_of_group[gg]
                            eng = nc.vector if (bcol % 2 == 0) else nc.gpsimd
                            eng.tensor_scalar(
                                out=s_t[:], in0=iota_b[:],
                                scalar1=rd_sb[:, lc:lc + 1], scalar2=None,
                                op0=mybir.AluOpType.is_equal)
                            slot = (bcol - blocks[0][0])
                            nc.tensor.matmul(out=pw[w], lhsT=s_t[:], rhs=g_t[:, slot, :],
                                             start=(first_block.get(w) == bcol),
                                             stop=(last_block.get(w) == bcol))

                    # epilogue, batched partition-major output DMA per group
                    o_g = sb.tile([128, WG * H], mybir.dt.float32, tag="o_g", bufs=2)
                    for w in range(wlo, whi):
                        u = sb.tile([128, H], mybir.dt.float32, tag="u", bufs=4)
                        nc.scalar.activation(u[:], pw[w], mybir.ActivationFunctionType.Copy,
                                             scale=dinv_sb[:, w:w + 1])
                        u2 = sb.tile([128, H], mybir.dt.float32, tag="u2", bufs=4)
                        nc.vector.tensor_tensor(out=u2[:], in0=u[:], in1=b128[:],
                                                op=mybir.AluOpType.add)
                        r2 = sb.tile([128, H], mybir.dt.float32, tag="r2", bufs=3)
                        nc.scalar.activation(r2[:], u2[:], mybir.ActivationFunctionType.Relu,
                                             scale=-1.0)
                        m = sb.tile([128, H], mybir.dt.float32, tag="m", bufs=3)
                        nc.gpsimd.tensor_tensor(out=m[:], in0=r2[:], in1=a128[:],
                                                op=mybir.AluOpType.mult)
                        r1 = sb.tile([128, H], mybir.dt.float32, tag="r1", bufs=3)
                        nc.scalar.activation(r1[:], u2[:], mybir.ActivationFunctionType.Relu)
                        nc.vector.tensor_tensor(out=o_g[:, (w - wlo) * H:(w - wlo + 1) * H],
                                                in0=r1[:], in1=m[:],
                                                op=mybir.AluOpType.subtract)
                    nc.scalar.dma_start(
                        out_d[:, wlo * H:whi * H],
                        o_g[:, 0:nwin * H])

    nc.compile()
    return nc


_LAST = {}


def kernel(x, edge_index, W, b, alpha):
    from concourse.bass_utils import run_bass_kernel_spmd

    x = np.asarray(x, dtype=np.float32)
    W = np.asarray(W, dtype=np.float32)
    b = np.asarray(b, dtype=np.float32)
    alpha = np.asarray(alpha, dtype=np.float32)

    sched, per_core, dinv_np = _preprocess(edge_index)
    nc = _build(sched)
    _LAST["nc"] = nc
    _LAST["sched"] = sched

    # x' = dinv * x, padded, K-major, columns permuted: col (g*4+c)*128 + p
    # holds node g*512 + p*4 + c  (so phase-1 writes have 1KB contiguity).
    x_pad = np.zeros((NP, KIN), np.float32)
    x_pad[:N] = dinv_np[:N, None] * x
    perm = np.arange(NP).reshape(NGRP, 128, 4).transpose(0, 2, 1).reshape(-1)
    # perm[(g*4+c)*128 + p] = g*512 + p*4 + c
    xt_perm = np.ascontiguousarray(x_pad[perm].T.astype(bf16))   # [256, NP]

    w_bf = W.astype(bf16)

    in_maps = []
    for c in range(NCORES):
        in_maps.append({
            "xt_perm": xt_perm,
            "w_bf": w_bf, "b_vec": b, "a_vec": alpha,
            "dinv_own": per_core[c]["dinv"],
            "idx16": per_core[c]["idx16"],
            "reldst": per_core[c]["reldst"],
        })

    res = run_bass_kernel_spmd(nc, in_maps, core_ids=list(range(NCORES)))
    # out_pm[d, w*H+f] -> rows w*128+d
    outs = []
    for c in range(NCORES):
        o = res.results[c]["out_pm"].reshape(128, NW, H).transpose(1, 0, 2)
        outs.append(o.reshape(SH, H))
    out = np.concatenate(outs, axis=0)
    return np.ascontiguousarray(out[:N])


# revision 6
# speedup vs baseline: 1.5078x; 1.1150x over previous
"""GCN layer (gather -> x@W -> normalized scatter-add -> bias -> PReLU) on 8 trn2 cores.

Strategy (no collectives; x replicated, full hs table computed locally per core):
  - 100000 nodes padded to 102400 = 8 * 12800; core c owns dst nodes [c*12800, (c+1)*12800).
  - Host: x' = dinv[:,None] * x (folds the src-side norm), padded, cast bf16,
    transposed to K-major [256, 102400], columns permuted so that phase-1 matmul
    chunk (g, c) yields, at out partition p, node g*512 + p*4 + c. Replicated to
    all 8 cores (51.2MB DMA each; cheaper in the cost model than any collective).
  - Phase 1: per 512-node group g: 8 matmuls (k=256 split in 2) into PSUM
    [128, 4*128], one Activation copy to bf16 SBUF, one DMA to the hs table with
    1KB-contiguous elements (4 consecutive rows per partition: row = g*512+p*4+c).
    Table stored as 4 quarter tensors [25600, 128] bf16 so gather idx fit int16.
  - Phase 3: edges (self-loops appended as real edges on host) sorted by
    (dst group gg of 8 windows, src quarter q, dst window); padding only per
    (gg, q) bin (to a 128 multiple), so 128-edge blocks may straddle one window
    boundary. Per block a one-hot S[e, d] = (iota_bank == rel[e]) is built on
    DVE, where rel is the dst offset from the block's first window (0..255,
    bf16-exact) and bank j compares against 128*j..128*j+127; PE accumulates
    psum[d, f] += S^T @ G per overlapped window, G = dma_gather'ed hs rows.
    Epilogue: out = prelu(dinv_dst * psum + b); output written partition-major
    [128, NW*H] (4KB contiguous) and transposed back on host.
"""
import sys
sys.path.insert(0, '/opt/trn_rl_repo')

import numpy as np
import ml_dtypes

N = 100000
NCORES = 8
SH = 12800                 # dst nodes per core
NP = NCORES * SH           # 102400 padded nodes
H = 128                    # output features
KIN = 256                  # input features
WIN = 128                  # dst window size
NW = SH // WIN             # 100 windows per core
WG = 8                     # windows per PSUM group
NG = (NW + WG - 1) // WG   # 13 groups (last has 4)
NQ = 4                     # source quarters (int16 gather idx: 25600 < 32768)
QTAB = NP // NQ            # 25600 rows per quarter table
GRP = 512                  # phase-1 rows per PSUM group
NGRP = NP // GRP           # 200 phase-1 groups

bf16 = ml_dtypes.bfloat16


def _preprocess(edge_index):
    e_src = np.asarray(edge_index[0]).astype(np.int64)
    e_dst = np.asarray(edge_index[1]).astype(np.int64)

    deg = (np.bincount(e_dst, minlength=N) + 1).astype(np.float32)
    dinv = (1.0 / np.sqrt(deg)).astype(np.float32)
    dinv_np = np.ones(NP, np.float32)
    dinv_np[:N] = dinv

    # self-loops as real edges (incl. padding nodes; their output is discarded)
    loops = np.arange(NP, dtype=np.int64)
    src = np.concatenate([e_src, loops])
    dst = np.concatenate([e_dst, loops])
    E = src.shape[0]

    core = dst // SH
    w_in_core = (dst % SH) // WIN            # 0..NW-1
    g = w_in_core // WG
    wi = w_in_core % WG
    q = src // QTAB                          # source quarter (canonical table order)
    tab_row = src % QTAB                     # row within quarter table

    # sort by (core, g, q, window)
    key = ((core * NG + g) * NQ + q) * WG + wi
    nbins_pc = NG * NQ * WG
    order = np.argsort(key, kind='stable')
    o_tab = tab_row[order]
    o_dst = dst[order]
    o_key = key[order]
    o_core = core[order]

    cnt_all = np.bincount(key, minlength=NCORES * nbins_pc)
    bin_start = np.concatenate([[0], np.cumsum(cnt_all)])[:-1]
    rank = np.arange(E, dtype=np.int64) - bin_start[o_key]   # within (c,g,q,w)

    # shared schedule: per (g,q,w) segment length = max count over cores
    # (edge granularity); each (g,q) bin padded to a 128 multiple once.
    maxcnt = cnt_all.reshape(NCORES, NG, NQ, WG).max(axis=0)  # [NG, NQ, WG]
    seg_end = np.cumsum(maxcnt, axis=2)
    wstart = seg_end - maxcnt                                 # [NG, NQ, WG]
    binlen = ((seg_end[:, :, -1] + WIN - 1) // WIN) * WIN     # [NG, NQ]
    offs_gq = np.concatenate([[0], np.cumsum(binlen.reshape(-1))])  # per (g,q)
    TOT = int(offs_gq[-1])
    NBLK = TOT // WIN

    gqw_in_core = o_key % nbins_pc
    gq_in_core = gqw_in_core // WG
    pos = (offs_gq[gq_in_core] + wstart.reshape(-1)[gqw_in_core] + rank)
    blkid = pos // WIN                                        # global block

    # per-block overlapped windows from the shared segment layout
    wmin_blk = np.zeros(NBLK, np.int64)
    blk_wins = [[] for _ in range(NBLK)]
    for gg in range(NG):
        for qq in range(NQ):
            bin_i = gg * NQ + qq
            b0 = int(offs_gq[bin_i]) // WIN
            nb = int(binlen[gg, qq]) // WIN
            for wii in range(WG):
                w = gg * WG + wii
                if w >= NW or maxcnt[gg, qq, wii] == 0:
                    continue
                lo = int(wstart[gg, qq, wii])
                hi = int(seg_end[gg, qq, wii])
                for b in range(b0 + lo // WIN, b0 + (hi - 1) // WIN + 1):
                    blk_wins[b].append(w)
            for b in range(b0, b0 + nb):
                assert len(blk_wins[b]) <= 2, "block spans >2 windows"
                wmin_blk[b] = blk_wins[b][0] if blk_wins[b] else 0

    rel = (o_dst - (o_core * SH + wmin_blk[blkid] * WIN)).astype(np.float32)
    assert rel.min() >= 0 and rel.max() <= 255.0

    per_core = []
    for c in range(NCORES):
        m = o_core == c
        idxq = np.zeros(TOT, np.int16)
        rels = np.full(TOT, -1.0, np.float32)
        p_c = pos[m]
        idxq[p_c] = o_tab[m].astype(np.int16)
        rels[p_c] = rel[m]
        idx16 = np.tile(np.ascontiguousarray(idxq.reshape(TOT // 16, 16).T), (8, 1))
        relm = np.ascontiguousarray(rels.reshape(NBLK, WIN).T)   # [128, NBLK]
        dinv_own = np.ascontiguousarray(
            dinv_np[c * SH:(c + 1) * SH].reshape(NW, WIN).T)     # [128, NW]
        per_core.append(dict(idx16=idx16, reldst=relm, dinv=dinv_own))

    # build calls: per (g, q): gather binlen[g,q] rows; per block the
    # matmul list [(Bcol, w, bank)] for each overlapped window
    calls = []          # (g, q, off_idx, nidx, [(Bcol, w, bank), ...])
    first_block_of_win = {}
    last_block_of_win = {}
    for gg in range(NG):
        for qq in range(NQ):
            nlen = int(binlen[gg, qq])
            if nlen == 0:
                continue
            bin_i = gg * NQ + qq
            off_idx = int(offs_gq[bin_i])
            b0 = off_idx // WIN
            mms = []
            for b in range(b0, b0 + nlen // WIN):
                for w in blk_wins[b]:
                    mms.append((b, w, w - blk_wins[b][0]))
                    first_block_of_win.setdefault(w, (b, w))
                    last_block_of_win[w] = (b, w)
            calls.append((gg, qq, off_idx, nlen, mms))
    sched = dict(calls=calls, first_block=first_block_of_win,
                 last_block=last_block_of_win, NBLK=NBLK, TOT=TOT)
    return sched, per_core, dinv_np


def _build(sched):
    from concourse import bass, bacc, tile, mybir

    nc = bacc.Bacc("TRN2", target_bir_lowering=False, debug=False,
                   enable_asserts=True, num_devices=NCORES)

    xt_d = nc.dram_tensor("xt_perm", [KIN, NP], mybir.dt.bfloat16, kind="ExternalInput")
    w_d = nc.dram_tensor("w_bf", [KIN, H], mybir.dt.bfloat16, kind="ExternalInput")
    b_d = nc.dram_tensor("b_vec", [H], mybir.dt.float32, kind="ExternalInput")
    a_d = nc.dram_tensor("a_vec", [H], mybir.dt.float32, kind="ExternalInput")
    dinv_d = nc.dram_tensor("dinv_own", [128, NW], mybir.dt.float32, kind="ExternalInput")
    idx_d = nc.dram_tensor("idx16", [128, sched["TOT"] // 16], mybir.dt.int16, kind="ExternalInput")
    rel_d = nc.dram_tensor("reldst", [128, sched["NBLK"]], mybir.dt.float32, kind="ExternalInput")

    # output in partition-major layout: out[d, w*H + f] = result[w*128 + d, f]
    out_d = nc.dram_tensor("out_pm", [128, NW * H], mybir.dt.float32, kind="ExternalOutput")

    hs_tab = [nc.dram_tensor(f"hs_tab{k}", [QTAB, H], mybir.dt.bfloat16) for k in range(NQ)]

    calls = sched["calls"]
    first_block = sched["first_block"]
    last_block = sched["last_block"]
    max_call_blk = max(cb[3] // WIN for cb in calls)
    nblk_of_group = [sum(cb[3] // WIN for cb in calls if cb[0] == gg) for gg in range(NG)]
    first_col_of_group = [min([cb[2] // WIN for cb in calls if cb[0] == gg] or [0])
                          for gg in range(NG)]

    with tile.TileContext(nc) as tc:
        with tc.tile_pool(name="consts", bufs=1) as cp, tc.tile_pool(name="sb", bufs=3) as sb:
            # ---------------- constants ----------------
            iota_bank = []
            for j in range(2):
                it = cp.tile([128, 128], mybir.dt.int32, tag=f"it{j}")
                nc.gpsimd.iota(it[:], pattern=[[1, 128]], base=j * 128,
                               channel_multiplier=0)
                ib = cp.tile([128, 128], mybir.dt.bfloat16, tag=f"ib{j}")
                nc.vector.tensor_copy(ib[:], it[:])
                iota_bank.append(ib)

            w0 = cp.tile([128, H], mybir.dt.bfloat16)
            w1 = cp.tile([128, H], mybir.dt.bfloat16)
            nc.sync.dma_start(w0[:], w_d[0:128, :])
            nc.sync.dma_start(w1[:], w_d[128:256, :])

            dinv_sb = cp.tile([128, NW], mybir.dt.float32)
            nc.sync.dma_start(dinv_sb[:], dinv_d[:, :])

            ones1 = cp.tile([1, H], mybir.dt.float32)
            nc.vector.memset(ones1[:], 1.0)
            bvec = cp.tile([1, H], mybir.dt.float32)
            nc.sync.dma_start(bvec[:], b_d[None, :])
            avec = cp.tile([1, H], mybir.dt.float32)
            nc.sync.dma_start(avec[:], a_d[None, :])

            b128 = cp.tile([128, H], mybir.dt.float32)
            a128 = cp.tile([128, H], mybir.dt.float32)

            with tc.tile_pool(name="psum1", bufs=1, space="PSUM") as pp1:
                bc_ps = pp1.tile([128, H], mybir.dt.float32, space="PSUM", tag="bc", bufs=1)
                nc.tensor.matmul(out=bc_ps[:], lhsT=ones1[:], rhs=bvec[:], start=True, stop=True)
                nc.vector.tensor_copy(b128[:], bc_ps[:])
                ac_ps = pp1.tile([128, H], mybir.dt.float32, space="PSUM", tag="bc", bufs=1)
                nc.tensor.matmul(out=ac_ps[:], lhsT=ones1[:], rhs=avec[:], start=True, stop=True)
                nc.vector.tensor_copy(a128[:], ac_ps[:])

                # ---------------- phase 1: full hs table, local ----------------
                for g in range(NGRP):
                    x_t = sb.tile([128, 2, GRP], mybir.dt.bfloat16, tag="x_t", bufs=3)
                    nc.sync.dma_start(
                        x_t[:],
                        xt_d[:, g * GRP:(g + 1) * GRP].rearrange(
                            "(a p) c -> p a c", p=128))
                    ps = pp1.tile([128, 4 * H], mybir.dt.float32, space="PSUM",
                                  tag="h_ps", bufs=3)
                    for cc in range(4):
                        for a in range(2):
                            nc.tensor.matmul(
                                out=ps[:, cc * H:(cc + 1) * H],
                                lhsT=x_t[:, a, cc * 128:(cc + 1) * 128],
                                rhs=(w0 if a == 0 else w1)[:],
                                start=(a == 0), stop=(a == 1))
                    hb = sb.tile([128, 4 * H], mybir.dt.bfloat16, tag="hb", bufs=3)
                    nc.scalar.activation(hb[:], ps[:],
                                         mybir.ActivationFunctionType.Copy)
                    qk, r0 = divmod(g * GRP, QTAB)
                    nc.scalar.dma_start(
                        hs_tab[qk][r0:r0 + GRP, :].rearrange("(p c) k -> p (c k)", c=4),
                        hb[:])

            # ---------------- phase 3 ----------------
            with tc.tile_pool(name="psum3", bufs=WG, space="PSUM") as pp3:
                for gg in range(NG):
                    wlo = gg * WG
                    whi = min(wlo + WG, NW)
                    nwin = whi - wlo
                    pw = {}
                    for w in range(wlo, whi):
                        pwt = pp3.tile([128, H], mybir.dt.float32, space="PSUM",
                                       tag="pw", name=f"pw{w}", bufs=8)
                        pw[w] = pwt[:]

                    rd_sb = sb.tile([128, max(nblk_of_group)], mybir.dt.float32,
                                    tag="rd", bufs=3)
                    c0 = first_col_of_group[gg]
                    nc.sync.dma_start(rd_sb[:, 0:nblk_of_group[gg]],
                                      rel_d[:, c0:c0 + nblk_of_group[gg]])

                    for (g_c, qq, off_idx, nidx, mms) in calls:
                        if g_c != gg:
                            continue
                        idx_sb = sb.tile([128, max_call_blk * 8], mybir.dt.int16,
                                         tag="idx", bufs=4)
                        nc.sync.dma_start(idx_sb[:, 0:nidx // 16],
                                          idx_d[:, off_idx // 16: (off_idx + nidx) // 16])
                        g_t = sb.tile([128, max_call_blk, H], mybir.dt.bfloat16,
                                      tag="g_t", bufs=6)
                        nc.gpsimd.dma_gather(
                            g_t[:, 0:nidx // 128, :], hs_tab[qq][:, :],
                            idx_sb[:, 0:nidx // 16], nidx, nidx, H,
                            single_packet=False)
                        b0 = off_idx // WIN
                        for (bcol, w, bank) in mms:
                            s_t = sb.tile([128, 128], mybir.dt.bfloat16, tag="s_t", bufs=12)
                            lc = bcol - first_col_of_group[gg]
                            nc.vector.tensor_scalar(
                                out=s_t[:], in0=iota_bank[bank][:],
                                scalar1=rd_sb[:, lc:lc + 1], scalar2=None,
                                op0=mybir.AluOpType.is_equal)
                            nc.tensor.matmul(out=pw[w], lhsT=s_t[:],
                                             rhs=g_t[:, bcol - b0, :],
                                             start=(first_block.get(w) == (bcol, w)),
                                             stop=(last_block.get(w) == (bcol, w)))

                    # epilogue, batched partition-major output DMA per group
                    o_g = sb.tile([128, WG * H], mybir.dt.float32, tag="o_g", bufs=2)
                    for w in range(wlo, whi):
                        u = sb.tile([128, H], mybir.dt.float32, tag="u", bufs=4)
                        nc.scalar.activation(u[:], pw[w], mybir.ActivationFunctionType.Copy,
                                             scale=dinv_sb[:, w:w + 1])
                        u2 = sb.tile([128, H], mybir.dt.float32, tag="u2", bufs=4)
                        nc.vector.tensor_tensor(out=u2[:], in0=u[:], in1=b128[:],
                                                op=mybir.AluOpType.add)
                        r2 = sb.tile([128, H], mybir.dt.float32, tag="r2", bufs=3)
                        nc.scalar.activation(r2[:], u2[:], mybir.ActivationFunctionType.Relu,
                                             scale=-1.0)
                        m = sb.tile([128, H], mybir.dt.float32, tag="m", bufs=3)
                        nc.gpsimd.tensor_tensor(out=m[:], in0=r2[:], in1=a128[:],
                                                op=mybir.AluOpType.mult)
                        r1 = sb.tile([128, H], mybir.dt.float32, tag="r1", bufs=3)
                        nc.scalar.activation(r1[:], u2[:], mybir.ActivationFunctionType.Relu)
                        nc.vector.tensor_tensor(out=o_g[:, (w - wlo) * H:(w - wlo + 1) * H],
                                                in0=r1[:], in1=m[:],
                                                op=mybir.AluOpType.subtract)
                    nc.scalar.dma_start(
                        out_d[:, wlo * H:whi * H],
                        o_g[:, 0:nwin * H])

    nc.compile()
    return nc


_LAST = {}


def kernel(x, edge_index, W, b, alpha):
    from concourse.bass_utils import run_bass_kernel_spmd

    x = np.asarray(x, dtype=np.float32)
    W = np.asarray(W, dtype=np.float32)
    b = np.asarray(b, dtype=np.float32)
    alpha = np.asarray(alpha, dtype=np.float32)

    sched, per_core, dinv_np = _preprocess(edge_index)
    nc = _build(sched)
    _LAST["nc"] = nc
    _LAST["sched"] = sched

    # x' = dinv * x, padded, K-major, columns permuted: col (g*4+c)*128 + p
    # holds node g*512 + p*4 + c  (so phase-1 writes have 1KB contiguity).
    x_pad = np.zeros((NP, KIN), np.float32)
    x_pad[:N] = dinv_np[:N, None] * x
    perm = np.arange(NP).reshape(NGRP, 128, 4).transpose(0, 2, 1).reshape(-1)
    # perm[(g*4+c)*128 + p] = g*512 + p*4 + c
    xt_perm = np.ascontiguousarray(x_pad[perm].T.astype(bf16))   # [256, NP]

    w_bf = W.astype(bf16)

    in_maps = []
    for c in range(NCORES):
        in_maps.append({
            "xt_perm": xt_perm,
            "w_bf": w_bf, "b_vec": b, "a_vec": alpha,
            "dinv_own": per_core[c]["dinv"],
            "idx16": per_core[c]["idx16"],
            "reldst": per_core[c]["reldst"],
        })

    res = run_bass_kernel_spmd(nc, in_maps, core_ids=list(range(NCORES)))
    # out_pm[d, w*H+f] -> rows w*128+d
    outs = []
    for c in range(NCORES):
        o = res.results[c]["out_pm"].reshape(128, NW, H).transpose(1, 0, 2)
        outs.append(o.reshape(SH, H))
    out = np.concatenate(outs, axis=0)
    return np.ascontiguousarray(out[:N])


# revision 9
# speedup vs baseline: 1.7589x; 1.1665x over previous
"""GCN layer (gather -> x@W -> normalized scatter-add -> bias -> PReLU) on 8 trn2 cores.

Strategy (no collectives; x replicated, full hs table computed locally per core):
  - 100000 nodes padded to 102400 = 8 * 12800; core c owns dst nodes [c*12800, (c+1)*12800).
  - Per-core rotated table layout: on core c, table slot t holds node
    (t + c*12800) % 102400 — realized purely via each core's host-built x input
    permutation. Self-loop edges (appended as real edges on host) then always
    hit quarter 0 on every core, keeping the shared schedule's max-over-cores
    padding small.
  - Host: x' = dinv[:,None] * x (folds the src-side norm), padded, cast bf16,
    K-major [256, 102400], columns permuted so phase-1 matmul chunk (g, c)
    yields, at out partition p, table slot g*512 + p*4 + c.
  - Phase 1: per 512-slot group: 8 matmuls (k=256 split in 2) into PSUM
    [128, 4*128], Activation copy to bf16 SBUF; x loads and table writes are
    batched 4 groups per DMA with 1KB+ contiguous elements. Table stored as 4
    quarter tensors [25600, 128] bf16 so gather idx fit int16.
  - Phase 3: edges sorted by (dst group gg of 8 windows, src quarter q, dst
    window); per (gg,q,window) segment length = max count over cores (edge
    granularity), each (gg,q) bin padded to 128 once, so 128-edge blocks may
    straddle one window boundary. Per block a one-hot S[e, d] =
    (iota_bank == rel[e]) is built on DVE (rel is dst offset from the block's
    first window, 0..255, bf16-exact; bank j covers 128j..128j+127); PE
    accumulates psum[d, f] += S^T @ G per overlapped window, G = dma_gather'ed
    hs rows. Epilogue: out = prelu(dinv_dst * psum + b) — a single Lrelu
    activation per window when b == 0 and alpha is uniform (true here);
    output written partition-major [128, NW*H] (4KB contiguous), transposed
    back on host.
"""
import sys
sys.path.insert(0, '/opt/trn_rl_repo')

import numpy as np
import ml_dtypes

N = 100000
NCORES = 8
SH = 12800                 # dst nodes per core
NP = NCORES * SH           # 102400 padded nodes
H = 128                    # output features
KIN = 256                  # input features
WIN = 128                  # dst window size
NW = SH // WIN             # 100 windows per core
WG = 8                     # windows per PSUM group
NG = (NW + WG - 1) // WG   # 13 groups (last has 4)
NQ = 4                     # source quarters (int16 gather idx: 25600 < 32768)
QTAB = NP // NQ            # 25600 rows per quarter table
GRP = 512                  # phase-1 rows per PSUM group
NGRP = NP // GRP           # 200 phase-1 groups
XB = 4                     # phase-1 groups per DMA batch

bf16 = ml_dtypes.bfloat16


def _preprocess(edge_index):
    e_src = np.asarray(edge_index[0]).astype(np.int64)
    e_dst = np.asarray(edge_index[1]).astype(np.int64)

    deg = (np.bincount(e_dst, minlength=N) + 1).astype(np.float32)
    dinv = (1.0 / np.sqrt(deg)).astype(np.float32)
    dinv_np = np.ones(NP, np.float32)
    dinv_np[:N] = dinv

    # self-loops as real edges (incl. padding nodes; their output is discarded)
    loops = np.arange(NP, dtype=np.int64)
    src = np.concatenate([e_src, loops])
    dst = np.concatenate([e_dst, loops])
    E = src.shape[0]

    core = dst // SH
    w_in_core = (dst % SH) // WIN            # 0..NW-1
    g = w_in_core // WG
    wi = w_in_core % WG
    rot = (src - core * SH) % NP             # per-core rotated table slot
    q = rot // QTAB                          # source quarter (self-loops -> q=0)
    tab_row = rot % QTAB                     # row within quarter table

    # sort by (core, g, q, window)
    key = ((core * NG + g) * NQ + q) * WG + wi
    nbins_pc = NG * NQ * WG
    order = np.argsort(key, kind='stable')
    o_tab = tab_row[order]
    o_dst = dst[order]
    o_key = key[order]
    o_core = core[order]

    cnt_all = np.bincount(key, minlength=NCORES * nbins_pc)
    bin_start = np.concatenate([[0], np.cumsum(cnt_all)])[:-1]
    rank = np.arange(E, dtype=np.int64) - bin_start[o_key]   # within (c,g,q,w)

    # shared schedule: per (g,q,w) segment length = max count over cores
    # (edge granularity); each (g,q) bin padded to a 128 multiple once.
    maxcnt = cnt_all.reshape(NCORES, NG, NQ, WG).max(axis=0)  # [NG, NQ, WG]
    seg_end = np.cumsum(maxcnt, axis=2)
    wstart = seg_end - maxcnt                                 # [NG, NQ, WG]
    binlen = ((seg_end[:, :, -1] + WIN - 1) // WIN) * WIN     # [NG, NQ]
    offs_gq = np.concatenate([[0], np.cumsum(binlen.reshape(-1))])  # per (g,q)
    TOT = int(offs_gq[-1])
    NBLK = TOT // WIN

    gqw_in_core = o_key % nbins_pc
    gq_in_core = gqw_in_core // WG
    pos = (offs_gq[gq_in_core] + wstart.reshape(-1)[gqw_in_core] + rank)
    blkid = pos // WIN                                        # global block

    # per-block overlapped windows from the shared segment layout
    wmin_blk = np.zeros(NBLK, np.int64)
    blk_wins = [[] for _ in range(NBLK)]
    for gg in range(NG):
        for qq in range(NQ):
            bin_i = gg * NQ + qq
            b0 = int(offs_gq[bin_i]) // WIN
            nb = int(binlen[gg, qq]) // WIN
            for wii in range(WG):
                w = gg * WG + wii
                if w >= NW or maxcnt[gg, qq, wii] == 0:
                    continue
                lo = int(wstart[gg, qq, wii])
                hi = int(seg_end[gg, qq, wii])
                for b in range(b0 + lo // WIN, b0 + (hi - 1) // WIN + 1):
                    blk_wins[b].append(w)
            for b in range(b0, b0 + nb):
                assert len(blk_wins[b]) <= 2, "block spans >2 windows"
                wmin_blk[b] = blk_wins[b][0] if blk_wins[b] else 0

    rel = (o_dst - (o_core * SH + wmin_blk[blkid] * WIN)).astype(np.float32)
    assert rel.min() >= 0 and rel.max() <= 255.0

    per_core = []
    for c in range(NCORES):
        m = o_core == c
        idxq = np.zeros(TOT, np.int16)
        rels = np.full(TOT, -1.0, np.float32)
        p_c = pos[m]
        idxq[p_c] = o_tab[m].astype(np.int16)
        rels[p_c] = rel[m]
        idx16 = np.tile(np.ascontiguousarray(idxq.reshape(TOT // 16, 16).T), (8, 1))
        relm = np.ascontiguousarray(rels.reshape(NBLK, WIN).T)   # [128, NBLK]
        dinv_own = np.ascontiguousarray(
            dinv_np[c * SH:(c + 1) * SH].reshape(NW, WIN).T)     # [128, NW]
        per_core.append(dict(idx16=idx16, reldst=relm, dinv=dinv_own))

    # build calls: per (g, q): gather binlen[g,q] rows; per block the
    # matmul list [(Bcol, w, bank)] for each overlapped window
    calls = []          # (g, q, off_idx, nidx, [(Bcol, w, bank), ...])
    first_block_of_win = {}
    last_block_of_win = {}
    for gg in range(NG):
        for qq in range(NQ):
            nlen = int(binlen[gg, qq])
            if nlen == 0:
                continue
            bin_i = gg * NQ + qq
            off_idx = int(offs_gq[bin_i])
            b0 = off_idx // WIN
            mms = []
            for b in range(b0, b0 + nlen // WIN):
                for w in blk_wins[b]:
                    mms.append((b, w, w - blk_wins[b][0]))
                    first_block_of_win.setdefault(w, (b, w))
                    last_block_of_win[w] = (b, w)
            calls.append((gg, qq, off_idx, nlen, mms))
    sched = dict(calls=calls, first_block=first_block_of_win,
                 last_block=last_block_of_win, NBLK=NBLK, TOT=TOT)
    return sched, per_core, dinv_np


def _build(sched, fast_epilogue=None):
    from concourse import bass, bacc, tile, mybir

    nc = bacc.Bacc("TRN2", target_bir_lowering=False, debug=False,
                   enable_asserts=True, num_devices=NCORES)

    xt_d = nc.dram_tensor("xt_perm", [KIN, NP], mybir.dt.bfloat16, kind="ExternalInput")
    w_d = nc.dram_tensor("w_bf", [KIN, H], mybir.dt.bfloat16, kind="ExternalInput")
    b_d = nc.dram_tensor("b_vec", [H], mybir.dt.float32, kind="ExternalInput")
    a_d = nc.dram_tensor("a_vec", [H], mybir.dt.float32, kind="ExternalInput")
    dinv_d = nc.dram_tensor("dinv_own", [128, NW], mybir.dt.float32, kind="ExternalInput")
    idx_d = nc.dram_tensor("idx16", [128, sched["TOT"] // 16], mybir.dt.int16, kind="ExternalInput")
    rel_d = nc.dram_tensor("reldst", [128, sched["NBLK"]], mybir.dt.float32, kind="ExternalInput")

    # output in partition-major layout: out[d, w*H + f] = result[w*128 + d, f]
    out_d = nc.dram_tensor("out_pm", [128, NW * H], mybir.dt.float32, kind="ExternalOutput")

    hs_tab = [nc.dram_tensor(f"hs_tab{k}", [QTAB, H], mybir.dt.bfloat16) for k in range(NQ)]

    calls = sched["calls"]
    first_block = sched["first_block"]
    last_block = sched["last_block"]
    max_call_blk = max(cb[3] // WIN for cb in calls)
    nblk_of_group = [sum(cb[3] // WIN for cb in calls if cb[0] == gg) for gg in range(NG)]
    first_col_of_group = [min([cb[2] // WIN for cb in calls if cb[0] == gg] or [0])
                          for gg in range(NG)]

    with tile.TileContext(nc) as tc:
        with tc.tile_pool(name="consts", bufs=1) as cp, tc.tile_pool(name="sb", bufs=3) as sb:
            # ---------------- constants ----------------
            iota_bank = []
            for j in range(2):
                it = cp.tile([128, 128], mybir.dt.int32, tag=f"it{j}")
                nc.gpsimd.iota(it[:], pattern=[[1, 128]], base=j * 128,
                               channel_multiplier=0)
                ib = cp.tile([128, 128], mybir.dt.bfloat16, tag=f"ib{j}")
                nc.vector.tensor_copy(ib[:], it[:])
                iota_bank.append(ib)

            w0 = cp.tile([128, H], mybir.dt.bfloat16)
            w1 = cp.tile([128, H], mybir.dt.bfloat16)
            nc.sync.dma_start(w0[:], w_d[0:128, :])
            nc.sync.dma_start(w1[:], w_d[128:256, :])

            dinv_sb = cp.tile([128, NW], mybir.dt.float32)
            nc.sync.dma_start(dinv_sb[:], dinv_d[:, :])

            ones1 = cp.tile([1, H], mybir.dt.float32)
            nc.vector.memset(ones1[:], 1.0)
            bvec = cp.tile([1, H], mybir.dt.float32)
            nc.sync.dma_start(bvec[:], b_d[None, :])
            avec = cp.tile([1, H], mybir.dt.float32)
            nc.sync.dma_start(avec[:], a_d[None, :])

            b128 = cp.tile([128, H], mybir.dt.float32)
            a128 = cp.tile([128, H], mybir.dt.float32)

            with tc.tile_pool(name="psum1", bufs=1, space="PSUM") as pp1:
                if not fast_epilogue:
                    bc_ps = pp1.tile([128, H], mybir.dt.float32, space="PSUM", tag="bc", bufs=1)
                    nc.tensor.matmul(out=bc_ps[:], lhsT=ones1[:], rhs=bvec[:], start=True, stop=True)
                    nc.vector.tensor_copy(b128[:], bc_ps[:])
                    ac_ps = pp1.tile([128, H], mybir.dt.float32, space="PSUM", tag="bc", bufs=1)
                    nc.tensor.matmul(out=ac_ps[:], lhsT=ones1[:], rhs=avec[:], start=True, stop=True)
                    nc.vector.tensor_copy(a128[:], ac_ps[:])

                # ---------------- phase 1: full hs table, local ----------------
                for bb in range(NGRP // XB):
                    g0 = bb * XB
                    x_t = sb.tile([128, 2, XB * GRP], mybir.dt.bfloat16, tag="x_t", bufs=3)
                    nc.sync.dma_start(
                        x_t[:],
                        xt_d[:, g0 * GRP:(g0 + XB) * GRP].rearrange(
                            "(a p) c -> p a c", p=128))
                    hb = sb.tile([128, XB, 4 * H], mybir.dt.bfloat16, tag="hb", bufs=2)
                    for gi in range(XB):
                        ps = pp1.tile([128, 4 * H], mybir.dt.float32, space="PSUM",
                                      tag="h_ps", bufs=3)
                        for cc in range(4):
                            for a in range(2):
                                nc.tensor.matmul(
                                    out=ps[:, cc * H:(cc + 1) * H],
                                    lhsT=x_t[:, a, (gi * 4 + cc) * 128:(gi * 4 + cc + 1) * 128],
                                    rhs=(w0 if a == 0 else w1)[:],
                                    start=(a == 0), stop=(a == 1))
                        nc.scalar.activation(hb[:, gi, :], ps[:],
                                             mybir.ActivationFunctionType.Copy)
                    # write XB groups; split at quarter boundaries
                    r_lo = g0 * GRP
                    while r_lo < (g0 + XB) * GRP:
                        qk = r_lo // QTAB
                        r_hi = min((g0 + XB) * GRP, (qk + 1) * QTAB)
                        gl = (r_lo - g0 * GRP) // GRP
                        gh = (r_hi - g0 * GRP) // GRP
                        nc.scalar.dma_start(
                            hs_tab[qk][r_lo - qk * QTAB:r_hi - qk * QTAB, :].rearrange(
                                "(gi p c) k -> p gi (c k)", p=128, c=4),
                            hb[:, gl:gh, :])
                        r_lo = r_hi

            # ---------------- phase 3 ----------------
            with tc.tile_pool(name="psum3", bufs=WG, space="PSUM") as pp3:
                for gg in range(NG):
                    wlo = gg * WG
                    whi = min(wlo + WG, NW)
                    nwin = whi - wlo
                    pw = {}
                    for w in range(wlo, whi):
                        pwt = pp3.tile([128, H], mybir.dt.float32, space="PSUM",
                                       tag="pw", name=f"pw{w}", bufs=8)
                        pw[w] = pwt[:]

                    rd_sb = sb.tile([128, max(nblk_of_group)], mybir.dt.float32,
                                    tag="rd", bufs=3)
                    c0 = first_col_of_group[gg]
                    nc.sync.dma_start(rd_sb[:, 0:nblk_of_group[gg]],
                                      rel_d[:, c0:c0 + nblk_of_group[gg]])

                    for (g_c, qq, off_idx, nidx, mms) in calls:
                        if g_c != gg:
                            continue
                        idx_sb = sb.tile([128, max_call_blk * 8], mybir.dt.int16,
                                         tag="idx", bufs=4)
                        nc.sync.dma_start(idx_sb[:, 0:nidx // 16],
                                          idx_d[:, off_idx // 16: (off_idx + nidx) // 16])
                        g_t = sb.tile([128, max_call_blk, H], mybir.dt.bfloat16,
                                      tag="g_t", bufs=6)
                        nc.gpsimd.dma_gather(
                            g_t[:, 0:nidx // 128, :], hs_tab[qq][:, :],
                            idx_sb[:, 0:nidx // 16], nidx, nidx, H,
                            single_packet=False)
                        b0 = off_idx // WIN
                        for (bcol, w, bank) in mms:
                            s_t = sb.tile([128, 128], mybir.dt.bfloat16, tag="s_t", bufs=12)
                            lc = bcol - first_col_of_group[gg]
                            nc.vector.tensor_scalar(
                                out=s_t[:], in0=iota_bank[bank][:],
                                scalar1=rd_sb[:, lc:lc + 1], scalar2=None,
                                op0=mybir.AluOpType.is_equal)
                            nc.tensor.matmul(out=pw[w], lhsT=s_t[:],
                                             rhs=g_t[:, bcol - b0, :],
                                             start=(first_block.get(w) == (bcol, w)),
                                             stop=(last_block.get(w) == (bcol, w)))

                    # epilogue, batched partition-major output DMA per group
                    o_g = sb.tile([128, WG * H], mybir.dt.float32, tag="o_g", bufs=2)
                    for w in range(wlo, whi):
                        og = o_g[:, (w - wlo) * H:(w - wlo + 1) * H]
                        if fast_epilogue:
                            # b == 0, uniform alpha<1: out = max(dinv*psum,
                            # alpha*dinv*psum)
                            al = float(fast_epilogue["alpha"])
                            u = sb.tile([128, H], mybir.dt.float32, tag="u", bufs=4)
                            nc.scalar.activation(u[:], pw[w],
                                                 mybir.ActivationFunctionType.Copy,
                                                 scale=dinv_sb[:, w:w + 1])
                            t = sb.tile([128, H], mybir.dt.float32, tag="t", bufs=4)
                            nc.vector.tensor_scalar(
                                out=t[:], in0=pw[w],
                                scalar1=dinv_sb[:, w:w + 1], scalar2=al,
                                op0=mybir.AluOpType.mult, op1=mybir.AluOpType.mult)
                            nc.vector.tensor_tensor(out=og, in0=u[:], in1=t[:],
                                                    op=mybir.AluOpType.max)
                        else:
                            u = sb.tile([128, H], mybir.dt.float32, tag="u", bufs=4)
                            nc.scalar.activation(u[:], pw[w],
                                                 mybir.ActivationFunctionType.Copy,
                                                 scale=dinv_sb[:, w:w + 1])
                            u2 = sb.tile([128, H], mybir.dt.float32, tag="u2", bufs=4)
                            nc.vector.tensor_tensor(out=u2[:], in0=u[:], in1=b128[:],
                                                    op=mybir.AluOpType.add)
                            r2 = sb.tile([128, H], mybir.dt.float32, tag="r2", bufs=3)
                            nc.scalar.activation(r2[:], u2[:],
                                                 mybir.ActivationFunctionType.Relu,
                                                 scale=-1.0)
                            m = sb.tile([128, H], mybir.dt.float32, tag="m", bufs=3)
                            nc.gpsimd.tensor_tensor(out=m[:], in0=r2[:], in1=a128[:],
                                                    op=mybir.AluOpType.mult)
                            r1 = sb.tile([128, H], mybir.dt.float32, tag="r1", bufs=3)
                            nc.scalar.activation(r1[:], u2[:],
                                                 mybir.ActivationFunctionType.Relu)
                            nc.vector.tensor_tensor(out=og,
                                                    in0=r1[:], in1=m[:],
                                                    op=mybir.AluOpType.subtract)
                    nc.scalar.dma_start(
                        out_d[:, wlo * H:whi * H],
                        o_g[:, 0:nwin * H])

    nc.compile()
    return nc


_LAST = {}


def kernel(x, edge_index, W, b, alpha):
    from concourse.bass_utils import run_bass_kernel_spmd

    x = np.asarray(x, dtype=np.float32)
    W = np.asarray(W, dtype=np.float32)
    b = np.asarray(b, dtype=np.float32)
    alpha = np.asarray(alpha, dtype=np.float32)

    sched, per_core, dinv_np = _preprocess(edge_index)
    fast = None
    if np.all(b == 0.0) and np.all(alpha == alpha.flat[0]) and 0.0 <= alpha.flat[0] <= 1.0:
        fast = {"alpha": float(alpha.flat[0])}
    nc = _build(sched, fast_epilogue=fast)
    _LAST["nc"] = nc
    _LAST["sched"] = sched

    # x' = dinv * x, padded; per-core: table slot t holds node (t + c*SH) % NP;
    # K-major columns permuted so col (g*4+cc)*128 + p holds slot g*512+p*4+cc.
    x_pad = np.zeros((NP, KIN), np.float32)
    x_pad[:N] = dinv_np[:N, None] * x
    slot_perm = np.arange(NP).reshape(NGRP, 128, 4).transpose(0, 2, 1).reshape(-1)
    # slot_perm[(g*4+cc)*128 + p] = g*512 + p*4 + cc

    w_bf = W.astype(bf16)

    in_maps = []
    for c in range(NCORES):
        node_of_col = (slot_perm + c * SH) % NP
        xt_c = np.ascontiguousarray(x_pad[node_of_col].T.astype(bf16))  # [256, NP]
        in_maps.append({
            "xt_perm": xt_c,
            "w_bf": w_bf, "b_vec": b, "a_vec": alpha,
            "dinv_own": per_core[c]["dinv"],
            "idx16": per_core[c]["idx16"],
            "reldst": per_core[c]["reldst"],
        })

    res = run_bass_kernel_spmd(nc, in_maps, core_ids=list(range(NCORES)))
    # out_pm[d, w*H+f] -> rows w*128+d
    outs = []
    for c in range(NCORES):
        o = res.results[c]["out_pm"].reshape(128, NW, H).transpose(1, 0, 2)
        outs.append(o.reshape(SH, H))
    out = np.concatenate(outs, axis=0)
    return np.ascontiguousarray(out[:N])


# revision 21
# speedup vs baseline: 1.9119x; 1.0870x over previous
"""GCN layer (gather -> x@W -> normalized scatter-add -> bias -> PReLU) on 8 trn2 cores.

Strategy (no collectives; x replicated, full hs table computed locally per core):
  - 100000 nodes padded to 102400 = 8 * 12800; core c owns dst nodes [c*12800, (c+1)*12800).
  - Per-core rotated table layout: on core c, table slot t holds node
    (t + c*12800) % 102400 — realized purely via each core's host-built x input
    permutation. Every core's own dst shard is then table slots [0, 12800),
    shared compile-time positions, and real cross-partition edges hit quarters
    uniformly, keeping the shared schedule's max-over-cores padding small.
  - Host: x' = dinv[:,None] * x (folds the src-side norm), padded, cast bf16,
    K-major [256, 102400], columns permuted so phase-1 matmul chunk (g, c)
    yields, at out partition p, table slot g*512 + p*4 + c.
  - Phase 1: per 512-slot group: 8 matmuls (k=256 split in 2) into PSUM
    [128, 4*128], Activation copy to bf16 SBUF; x loads and table writes are
    batched 4 groups per DMA with 1KB+ contiguous elements. Table stored as 4
    quarter tensors [25600, 128] bf16 so gather idx fit int16. Groups 0-24
    (own dst shard) are kept resident in SBUF (hs_own) as well.
  - Phase 3: self-loop contributions come from hs_own via 4 constant
    selection-matrix matmuls per window (no gather rows spent on them).
    Real edges sorted by (dst group gg of 8 windows, src quarter q, dst
    window); per (gg,q,w) segment length = max count over cores (edge
    granularity), each (gg,q) bin padded to 128 once, so 128-edge blocks may
    straddle one window boundary. Per block a one-hot S[e, d] =
    (iota_bank == rel[e]) is built on DVE (rel is dst offset from the block's
    first window, 0..255, bf16-exact; bank j covers 128j..128j+127); PE
    accumulates psum[d, f] += S^T @ G per overlapped window, G = dma_gather'ed
    hs rows (idx streams stored 16-partition, not replicated). Epilogue:
    out = prelu(dinv_dst * psum + b) as max(u, alpha*u) when b == 0 and alpha
    uniform in [0,1] (true here); bf16 output written partition-major
    [128, NW*H], upcast + transposed back on host.
"""
import sys
sys.path.insert(0, '/opt/trn_rl_repo')

import numpy as np
import ml_dtypes

N = 100000
NCORES = 8
SH = 12800                 # dst nodes per core
NP = NCORES * SH           # 102400 padded nodes
H = 128                    # output features
KIN = 256                  # input features
WIN = 128                  # dst window size
NW = SH // WIN             # 100 windows per core
WG = 8                     # windows per PSUM group
NG = (NW + WG - 1) // WG   # 13 groups (last has 4)
NQ = 4                     # source quarters (int16 gather idx: 25600 < 32768)
QTAB = NP // NQ            # 25600 rows per quarter table
GRP = 512                  # phase-1 rows per PSUM group
NGRP = NP // GRP           # 200 phase-1 groups
XB = 4                     # phase-1 groups per DMA batch
NOWN = SH // GRP           # 25 groups resident in SBUF (own dst shard)
PREF = 4                   # idx loads prefetched ahead of phase 3

bf16 = ml_dtypes.bfloat16


def _preprocess(edge_index):
    e_src = np.asarray(edge_index[0]).astype(np.int64)
    e_dst = np.asarray(edge_index[1]).astype(np.int64)

    deg = (np.bincount(e_dst, minlength=N) + 1).astype(np.float32)
    dinv = (1.0 / np.sqrt(deg)).astype(np.float32)
    dinv_np = np.ones(NP, np.float32)
    dinv_np[:N] = dinv

    # real edges only; self-loops are applied from SBUF in phase 3
    src = e_src
    dst = e_dst
    E = src.shape[0]

    core = dst // SH
    w_in_core = (dst % SH) // WIN            # 0..NW-1
    g = w_in_core // WG
    wi = w_in_core % WG
    rot = (src - core * SH) % NP             # per-core rotated table slot
    q = rot // QTAB                          # source quarter
    tab_row = rot % QTAB                     # row within quarter table

    # sort by (core, g, q, window)
    key = ((core * NG + g) * NQ + q) * WG + wi
    nbins_pc = NG * NQ * WG
    order = np.argsort(key, kind='stable')
    o_tab = tab_row[order]
    o_dst = dst[order]
    o_key = key[order]
    o_core = core[order]

    cnt_all = np.bincount(key, minlength=NCORES * nbins_pc)
    bin_start = np.concatenate([[0], np.cumsum(cnt_all)])[:-1]
    rank = np.arange(E, dtype=np.int64) - bin_start[o_key]   # within (c,g,q,w)

    # every window must have at least one real edge on SOME core, so the
    # shared schedule has a stop-flag anchor for each PSUM region
    wcnt = cnt_all.reshape(NCORES, NG, NQ, WG).sum(axis=(0, 2)).reshape(-1)
    assert wcnt[:NW].min() >= 1

    # shared schedule: per (g,q,w) segment length = max count over cores
    # (edge granularity); each (g,q) bin padded to a 128 multiple once.
    maxcnt = cnt_all.reshape(NCORES, NG, NQ, WG).max(axis=0)  # [NG, NQ, WG]
    seg_end = np.cumsum(maxcnt, axis=2)
    wstart = seg_end - maxcnt                                 # [NG, NQ, WG]
    binlen = ((seg_end[:, :, -1] + WIN - 1) // WIN) * WIN     # [NG, NQ]
    offs_gq = np.concatenate([[0], np.cumsum(binlen.reshape(-1))])  # per (g,q)
    TOT = int(offs_gq[-1])
    NBLK = TOT // WIN

    gqw_in_core = o_key % nbins_pc
    gq_in_core = gqw_in_core // WG
    pos = (offs_gq[gq_in_core] + wstart.reshape(-1)[gqw_in_core] + rank)
    blkid = pos // WIN                                        # global block

    # per-block overlapped windows from the shared segment layout
    wmin_blk = np.zeros(NBLK, np.int64)
    blk_wins = [[] for _ in range(NBLK)]
    for gg in range(NG):
        for qq in range(NQ):
            bin_i = gg * NQ + qq
            b0 = int(offs_gq[bin_i]) // WIN
            nb = int(binlen[gg, qq]) // WIN
            for wii in range(WG):
                w = gg * WG + wii
                if w >= NW or maxcnt[gg, qq, wii] == 0:
                    continue
                lo = int(wstart[gg, qq, wii])
                hi = int(seg_end[gg, qq, wii])
                for b in range(b0 + lo // WIN, b0 + (hi - 1) // WIN + 1):
                    blk_wins[b].append(w)
            for b in range(b0, b0 + nb):
                assert len(blk_wins[b]) <= 2, "block spans >2 windows"
                wmin_blk[b] = blk_wins[b][0] if blk_wins[b] else 0

    rel = (o_dst - (o_core * SH + wmin_blk[blkid] * WIN)).astype(np.float32)
    assert rel.min() >= 0 and rel.max() <= 255.0

    per_core = []
    for c in range(NCORES):
        m = o_core == c
        idxq = np.zeros(TOT, np.int16)
        rels = np.full(TOT, -1.0, np.float32)
        p_c = pos[m]
        idxq[p_c] = o_tab[m].astype(np.int16)
        rels[p_c] = rel[m]
        idx16 = np.tile(np.ascontiguousarray(idxq.reshape(TOT // 16, 16).T), (8, 1))
        relm = np.ascontiguousarray(rels.reshape(NBLK, WIN).T)   # [128, NBLK]
        dinv_own = np.ascontiguousarray(
            dinv_np[c * SH:(c + 1) * SH].reshape(NW, WIN).T)     # [128, NW]
        per_core.append(dict(idx16=idx16, reldst=relm, dinv=dinv_own))

    # build calls: per (g, q): gather binlen[g,q] rows; per block the
    # matmul list [(Bcol, w, bank)] for each overlapped window
    calls = []          # (g, q, off_idx, nidx, [(Bcol, w, bank), ...])
    last_block_of_win = {}
    for gg in range(NG):
        for qq in range(NQ):
            nlen = int(binlen[gg, qq])
            if nlen == 0:
                continue
            bin_i = gg * NQ + qq
            off_idx = int(offs_gq[bin_i])
            b0 = off_idx // WIN
            mms = []
            for b in range(b0, b0 + nlen // WIN):
                for w in blk_wins[b]:
                    mms.append((b, w, w - blk_wins[b][0]))
                    last_block_of_win[w] = (b, w)
            calls.append((gg, qq, off_idx, nlen, mms))
    sched = dict(calls=calls, last_block=last_block_of_win,
                 NBLK=NBLK, TOT=TOT)
    return sched, per_core, dinv_np


def _build(sched, fast_epilogue=None, self_mode="sel"):
    from concourse import bass, bacc, tile, mybir

    nc = bacc.Bacc("TRN2", target_bir_lowering=False, debug=False,
                   enable_asserts=True, num_devices=NCORES)

    xt_d = nc.dram_tensor("xt_perm", [KIN, NP], mybir.dt.bfloat16, kind="ExternalInput")
    w_d = nc.dram_tensor("w_bf", [KIN, H], mybir.dt.bfloat16, kind="ExternalInput")
    b_d = nc.dram_tensor("b_vec", [H], mybir.dt.float32, kind="ExternalInput")
    a_d = nc.dram_tensor("a_vec", [H], mybir.dt.float32, kind="ExternalInput")
    dinv_d = nc.dram_tensor("dinv_own", [128, NW], mybir.dt.float32, kind="ExternalInput")
    idx_d = nc.dram_tensor("idx16", [128, sched["TOT"] // 16], mybir.dt.int16, kind="ExternalInput")
    rel_d = nc.dram_tensor("reldst", [128, sched["NBLK"]], mybir.dt.float32, kind="ExternalInput")

    # output in partition-major layout: out[d, w*H + f] = result[w*128 + d, f]
    out_d = nc.dram_tensor("out_pm", [128, NW * H], mybir.dt.bfloat16, kind="ExternalOutput")

    hs_tab = [nc.dram_tensor(f"hs_tab{k}", [QTAB, H], mybir.dt.bfloat16) for k in range(NQ)]

    calls = sched["calls"]
    last_block = sched["last_block"]
    first_block = {}
    for (_gg, _qq, _oi, _ni, _mms) in calls:
        for (b, w, _bank) in _mms:
            first_block.setdefault(w, (b, w))
    max_call_blk = max(cb[3] // WIN for cb in calls)
    nblk_of_group = [sum(cb[3] // WIN for cb in calls if cb[0] == gg) for gg in range(NG)]
    first_col_of_group = [min([cb[2] // WIN for cb in calls if cb[0] == gg] or [0])
                          for gg in range(NG)]

    with tile.TileContext(nc) as tc:
        with tc.tile_pool(name="consts", bufs=1) as cp, tc.tile_pool(name="sb", bufs=3) as sb:
            # ---------------- constants ----------------
            iota_bank = []
            for j in range(2):
                it = cp.tile([128, 128], mybir.dt.int32, tag=f"it{j}")
                nc.gpsimd.iota(it[:], pattern=[[1, 128]], base=j * 128,
                               channel_multiplier=0)
                ib = cp.tile([128, 128], mybir.dt.bfloat16, tag=f"ib{j}")
                nc.vector.tensor_copy(ib[:], it[:])
                iota_bank.append(ib)

            # selection matrices for self-loop rows: sel[r][c][p, d] = 1 iff
            # d == 4p - 128r + c  (window w = 4*g_own + r, chunk c)
            selmat = []
            for r in range(4):
                row = []
                for c in range(4):
                    cv = cp.tile([128, 1], mybir.dt.int32, tag=f"cv{r}{c}")
                    nc.gpsimd.iota(cv[:], pattern=[[0, 1]], base=c - 128 * r,
                                   channel_multiplier=4)
                    cvf = cp.tile([128, 1], mybir.dt.float32, tag=f"cvf{r}{c}")
                    nc.vector.tensor_copy(cvf[:], cv[:])
                    sm = cp.tile([128, 128], mybir.dt.bfloat16, tag=f"sm{r}{c}")
                    nc.vector.tensor_scalar(
                        out=sm[:], in0=iota_bank[0][:],
                        scalar1=cvf[:], scalar2=None,
                        op0=mybir.AluOpType.is_equal)
                    row.append(sm)
                selmat.append(row)

            w0 = cp.tile([128, H], mybir.dt.bfloat16)
            w1 = cp.tile([128, H], mybir.dt.bfloat16)
            nc.sync.dma_start(w0[:], w_d[0:128, :])
            nc.sync.dma_start(w1[:], w_d[128:256, :])

            dinv_sb = cp.tile([128, NW], mybir.dt.float32)
            nc.sync.dma_start(dinv_sb[:], dinv_d[:, :])

            ones1 = cp.tile([1, H], mybir.dt.float32)
            nc.vector.memset(ones1[:], 1.0)
            bvec = cp.tile([1, H], mybir.dt.float32)
            nc.sync.dma_start(bvec[:], b_d[None, :])
            avec = cp.tile([1, H], mybir.dt.float32)
            nc.sync.dma_start(avec[:], a_d[None, :])

            b128 = cp.tile([128, H], mybir.dt.float32)
            a128 = cp.tile([128, H], mybir.dt.float32)

            hs_own = cp.tile([128, NOWN, GRP], mybir.dt.bfloat16)  # 25KB/part

            with tc.tile_pool(name="psum1", bufs=1, space="PSUM") as pp1:
                if not fast_epilogue:
                    bc_ps = pp1.tile([128, H], mybir.dt.float32, space="PSUM", tag="bc", bufs=1)
                    nc.tensor.matmul(out=bc_ps[:], lhsT=ones1[:], rhs=bvec[:], start=True, stop=True)
                    nc.vector.tensor_copy(b128[:], bc_ps[:])
                    ac_ps = pp1.tile([128, H], mybir.dt.float32, space="PSUM", tag="bc", bufs=1)
                    nc.tensor.matmul(out=ac_ps[:], lhsT=ones1[:], rhs=avec[:], start=True, stop=True)
                    nc.vector.tensor_copy(a128[:], ac_ps[:])

                # ---------------- phase 1: full hs table, local ----------------
                for bb in range(NGRP // XB):
                    g0 = bb * XB
                    x_t = sb.tile([128, 2, XB * GRP], mybir.dt.bfloat16, tag="x_t", bufs=3)
                    nc.sync.dma_start(
                        x_t[:],
                        xt_d[:, g0 * GRP:(g0 + XB) * GRP].rearrange(
                            "(a p) c -> p a c", p=128))
                    hb = sb.tile([128, XB, 4 * H], mybir.dt.bfloat16, tag="hb", bufs=2)
                    for gi in range(XB):
                        g = g0 + gi
                        ps = pp1.tile([128, 4 * H], mybir.dt.float32, space="PSUM",
                                      tag="h_ps", bufs=3)
                        for cc in range(4):
                            for a in range(2):
                                nc.tensor.matmul(
                                    out=ps[:, cc * H:(cc + 1) * H],
                                    lhsT=x_t[:, a, (gi * 4 + cc) * 128:(gi * 4 + cc + 1) * 128],
                                    rhs=(w0 if a == 0 else w1)[:],
                                    start=(a == 0), stop=(a == 1))
                        nc.scalar.activation(hb[:, gi, :], ps[:],
                                             mybir.ActivationFunctionType.Copy)
                        if g < NOWN:
                            nc.vector.tensor_copy(hs_own[:, g, :], ps[:])
                    # write XB groups; split at quarter boundaries
                    r_lo = g0 * GRP
                    while r_lo < (g0 + XB) * GRP:
                        qk = r_lo // QTAB
                        r_hi = min((g0 + XB) * GRP, (qk + 1) * QTAB)
                        gl = (r_lo - g0 * GRP) // GRP
                        gh = (r_hi - g0 * GRP) // GRP
                        dview = hs_tab[qk][r_lo - qk * QTAB:r_hi - qk * QTAB, :]
                        if gh - gl == 1:
                            nc.scalar.dma_start(
                                dview.rearrange("(p c) k -> p (c k)", p=128, c=4),
                                hb[:, gl, :])
                        else:
                            nc.scalar.dma_start(
                                dview.rearrange("(gi p c) k -> p gi (c k)", p=128, c=4),
                                hb[:, gl:gh, :])
                        r_lo = r_hi

            # ---------------- phase 3 ----------------
            with tc.tile_pool(name="psum3", bufs=WG, space="PSUM") as pp3:
                # prefetch first idx streams
                idx_tiles = {}
                for ci in range(min(PREF, len(calls))):
                    (gg0, qq0, off_idx, nidx, _mm) = calls[ci]
                    idx_sb = sb.tile([128, max_call_blk * 8], mybir.dt.int16,
                                     tag="idx", bufs=PREF + 1)
                    nc.sync.dma_start(idx_sb[:, 0:nidx // 16],
                                      idx_d[:, off_idx // 16: (off_idx + nidx) // 16])
                    idx_tiles[ci] = idx_sb

                for gg in range(NG):
                    wlo = gg * WG
                    whi = min(wlo + WG, NW)
                    nwin = whi - wlo
                    pw = {}
                    for w in range(wlo, whi):
                        pwt = pp3.tile([128, H], mybir.dt.float32, space="PSUM",
                                       tag="pw", name=f"pw{w}", bufs=8)
                        pw[w] = pwt[:]
                        if self_mode == "sel":
                            # self-loop contribution from SBUF-resident own rows
                            r = w % 4
                            for c in range(4):
                                nc.tensor.matmul(
                                    out=pw[w], lhsT=selmat[r][c][:],
                                    rhs=hs_own[:, w // 4, c * 128:(c + 1) * 128],
                                    start=(c == 0), stop=False)

                    rd_sb = sb.tile([128, max(nblk_of_group)], mybir.dt.float32,
                                    tag="rd", bufs=3)
                    c0 = first_col_of_group[gg]
                    nc.sync.dma_start(rd_sb[:, 0:nblk_of_group[gg]],
                                      rel_d[:, c0:c0 + nblk_of_group[gg]])

                    for ci, (g_c, qq, off_idx, nidx, mms) in enumerate(calls):
                        if g_c != gg:
                            continue
                        if ci in idx_tiles:
                            idx_sb = idx_tiles.pop(ci)
                        else:
                            idx_sb = sb.tile([128, max_call_blk * 8], mybir.dt.int16,
                                             tag="idx", bufs=PREF + 1)
                            nc.sync.dma_start(idx_sb[:, 0:nidx // 16],
                                              idx_d[:, off_idx // 16: (off_idx + nidx) // 16])
                        # prefetch a later idx stream to stay ahead
                        cn = ci + PREF
                        if cn < len(calls) and cn not in idx_tiles:
                            (_g2, _q2, off2, nidx2, _m2) = calls[cn]
                            nx = sb.tile([128, max_call_blk * 8], mybir.dt.int16,
                                         tag="idx", bufs=PREF + 1)
                            nc.sync.dma_start(nx[:, 0:nidx2 // 16],
                                              idx_d[:, off2 // 16: (off2 + nidx2) // 16])
                            idx_tiles[cn] = nx
                        g_t = sb.tile([128, max_call_blk, H], mybir.dt.bfloat16,
                                      tag="g_t", bufs=6)
                        nc.gpsimd.dma_gather(
                            g_t[:, 0:nidx // 128, :], hs_tab[qq][:, :],
                            idx_sb[:, 0:nidx // 16], nidx, nidx, H,
                            single_packet=False)
                        b0 = off_idx // WIN
                        for (bcol, w, bank) in mms:
                            s_t = sb.tile([128, 128], mybir.dt.bfloat16, tag="s_t", bufs=12)
                            lc = bcol - first_col_of_group[gg]
                            nc.vector.tensor_scalar(
                                out=s_t[:], in0=iota_bank[bank][:],
                                scalar1=rd_sb[:, lc:lc + 1], scalar2=None,
                                op0=mybir.AluOpType.is_equal)
                            nc.tensor.matmul(out=pw[w], lhsT=s_t[:],
                                             rhs=g_t[:, bcol - b0, :],
                                             start=(self_mode != "sel"
                                                    and first_block.get(w) == (bcol, w)),
                                             stop=(last_block.get(w) == (bcol, w)))

                    # epilogue, batched partition-major output DMA per group
                    o_g = sb.tile([128, WG * H], mybir.dt.bfloat16, tag="o_g", bufs=2)
                    for w in range(wlo, whi):
                        og = o_g[:, (w - wlo) * H:(w - wlo + 1) * H]
                        if fast_epilogue:
                            # b == 0, uniform alpha<=1: out = max(dinv*psum,
                            # alpha*dinv*psum)
                            al = float(fast_epilogue["alpha"])
                            u = sb.tile([128, H], mybir.dt.float32, tag="u", bufs=4)
                            nc.scalar.activation(u[:], pw[w],
                                                 mybir.ActivationFunctionType.Copy,
                                                 scale=dinv_sb[:, w:w + 1])
                            t = sb.tile([128, H], mybir.dt.float32, tag="t", bufs=4)
                            nc.vector.tensor_scalar(
                                out=t[:], in0=pw[w],
                                scalar1=dinv_sb[:, w:w + 1], scalar2=al,
                                op0=mybir.AluOpType.mult, op1=mybir.AluOpType.mult)
                            nc.vector.tensor_tensor(out=og, in0=u[:], in1=t[:],
                                                    op=mybir.AluOpType.max)
                        else:
                            u = sb.tile([128, H], mybir.dt.float32, tag="u", bufs=4)
                            nc.scalar.activation(u[:], pw[w],
                                                 mybir.ActivationFunctionType.Copy,
                                                 scale=dinv_sb[:, w:w + 1])
                            u2 = sb.tile([128, H], mybir.dt.float32, tag="u2", bufs=4)
                            nc.vector.tensor_tensor(out=u2[:], in0=u[:], in1=b128[:],
                                                    op=mybir.AluOpType.add)
                            r2 = sb.tile([128, H], mybir.dt.float32, tag="r2", bufs=3)
                            nc.scalar.activation(r2[:], u2[:],
                                                 mybir.ActivationFunctionType.Relu,
                                                 scale=-1.0)
                            m = sb.tile([128, H], mybir.dt.float32, tag="m", bufs=3)
                            nc.gpsimd.tensor_tensor(out=m[:], in0=r2[:], in1=a128[:],
                                                    op=mybir.AluOpType.mult)
                            r1 = sb.tile([128, H], mybir.dt.float32, tag="r1", bufs=3)
                            nc.scalar.activation(r1[:], u2[:],
                                                 mybir.ActivationFunctionType.Relu)
                            nc.vector.tensor_tensor(out=og,
                                                    in0=r1[:], in1=m[:],
                                                    op=mybir.AluOpType.subtract)
                    nc.scalar.dma_start(
                        out_d[:, wlo * H:whi * H],
                        o_g[:, 0:nwin * H])

    nc.compile()
    return nc


_LAST = {}


def kernel(x, edge_index, W, b, alpha):
    from concourse.bass_utils import run_bass_kernel_spmd

    x = np.asarray(x, dtype=np.float32)
    W = np.asarray(W, dtype=np.float32)
    b = np.asarray(b, dtype=np.float32)
    alpha = np.asarray(alpha, dtype=np.float32)

    sched, per_core, dinv_np = _preprocess(edge_index)
    fast = None
    if np.all(b == 0.0) and np.all(alpha == alpha.flat[0]) and 0.0 <= alpha.flat[0] <= 1.0:
        fast = {"alpha": float(alpha.flat[0])}
    nc = _build(sched, fast_epilogue=fast)
    _LAST["nc"] = nc
    _LAST["sched"] = sched

    # x' = dinv * x, padded; per-core: table slot t holds node (t + c*SH) % NP;
    # K-major columns permuted so col (g*4+cc)*128 + p holds slot g*512+p*4+cc.
    x_pad = np.zeros((NP, KIN), np.float32)
    x_pad[:N] = dinv_np[:N, None] * x
    slot_perm = np.arange(NP).reshape(NGRP, 128, 4).transpose(0, 2, 1).reshape(-1)
    # slot_perm[(g*4+cc)*128 + p] = g*512 + p*4 + cc

    w_bf = W.astype(bf16)

    in_maps = []
    for c in range(NCORES):
        node_of_col = (slot_perm + c * SH) % NP
        xt_c = np.ascontiguousarray(x_pad[node_of_col].T.astype(bf16))  # [256, NP]
        in_maps.append({
            "xt_perm": xt_c,
            "w_bf": w_bf, "b_vec": b, "a_vec": alpha,
            "dinv_own": per_core[c]["dinv"],
            "idx16": per_core[c]["idx16"],
            "reldst": per_core[c]["reldst"],
        })

    res = run_bass_kernel_spmd(nc, in_maps, core_ids=list(range(NCORES)))
    # out_pm[d, w*H+f] -> rows w*128+d
    outs = []
    for c in range(NCORES):
        o = res.results[c]["out_pm"].astype(np.float32).reshape(128, NW, H).transpose(1, 0, 2)
        outs.append(o.reshape(SH, H))
    out = np.concatenate(outs, axis=0)
    return np.ascontiguousarray(out[:N])


# revision 23
# speedup vs baseline: 1.9332x; 1.0112x over previous
"""GCN layer (gather -> x@W -> normalized scatter-add -> bias -> PReLU) on 8 trn2 cores.

Strategy (no collectives; x replicated, full hs table computed locally per core):
  - 100000 nodes padded to 102400 = 8 * 12800; core c owns dst nodes [c*12800, (c+1)*12800).
  - Per-core rotated table layout: on core c, table slot t holds node
    (t + c*12800) % 102400 — realized purely via each core's host-built x input
    permutation. Every core's own dst shard is then table slots [0, 12800),
    shared compile-time positions, and real cross-partition edges hit quarters
    uniformly, keeping the shared schedule's max-over-cores padding small.
  - Host: x' = dinv[:,None] * x (folds the src-side norm), padded, cast bf16,
    K-major [256, 102400], columns permuted so phase-1 matmul chunk (g, c)
    yields, at out partition p, table slot g*512 + p*4 + c.
  - Phase 1: per 512-slot group: 8 matmuls (k=256 split in 2) into PSUM
    [128, 4*128], Activation copy to bf16 SBUF; x loads and table writes are
    batched 4 groups per DMA with 1KB+ contiguous elements. Table stored as 4
    quarter tensors [25600, 128] bf16 so gather idx fit int16. Groups 0-24
    (own dst shard) are kept resident in SBUF (hs_own) as well.
  - Phase 3: self-loop contributions come from hs_own via 4 constant
    selection-matrix matmuls per window (no gather rows spent on them).
    Real edges sorted by (dst group gg of 8 windows, src quarter q, dst
    window); per (gg,q,w) segment length = max count over cores (edge
    granularity), each (gg,q) bin padded to 128 once, so 128-edge blocks may
    straddle one window boundary. Per block a one-hot S[e, d] =
    (iota_bank == rel[e]) is built on DVE (rel is dst offset from the block's
    first window, 0..255, bf16-exact; bank j covers 128j..128j+127); PE
    accumulates psum[d, f] += S^T @ G per overlapped window, G = dma_gather'ed
    hs rows (idx streams stored 16-partition, not replicated). Epilogue:
    out = prelu(dinv_dst * psum + b) as max(u, alpha*u) when b == 0 and alpha
    uniform in [0,1] (true here); bf16 output written partition-major
    [128, NW*H], upcast + transposed back on host.
"""
import sys
sys.path.insert(0, '/opt/trn_rl_repo')

import numpy as np
import ml_dtypes

N = 100000
NCORES = 8
SH = 12800                 # dst nodes per core
NP = NCORES * SH           # 102400 padded nodes
H = 128                    # output features
KIN = 256                  # input features
WIN = 128                  # dst window size
NW = SH // WIN             # 100 windows per core
WG = 8                     # windows per PSUM group
NG = (NW + WG - 1) // WG   # 13 groups (last has 4)
NQ = 4                     # source quarters (int16 gather idx: 25600 < 32768)
QTAB = NP // NQ            # 25600 rows per quarter table
GRP = 512                  # phase-1 rows per PSUM group
NGRP = NP // GRP           # 200 phase-1 groups
XB = 4                     # phase-1 groups per DMA batch
NOWN = SH // GRP           # 25 groups resident in SBUF (own dst shard)
PREF = 4                   # idx loads prefetched ahead of phase 3

bf16 = ml_dtypes.bfloat16


def _preprocess(edge_index):
    e_src = np.asarray(edge_index[0]).astype(np.int64)
    e_dst = np.asarray(edge_index[1]).astype(np.int64)

    deg = (np.bincount(e_dst, minlength=N) + 1).astype(np.float32)
    dinv = (1.0 / np.sqrt(deg)).astype(np.float32)
    dinv_np = np.ones(NP, np.float32)
    dinv_np[:N] = dinv

    # real edges only; self-loops are applied from SBUF in phase 3
    src = e_src
    dst = e_dst
    E = src.shape[0]

    core = dst // SH
    w_in_core = (dst % SH) // WIN            # 0..NW-1
    g = w_in_core // WG
    wi = w_in_core % WG
    rot = (src - core * SH) % NP             # per-core rotated table slot
    q = rot // QTAB                          # source quarter
    tab_row = rot % QTAB                     # row within quarter table

    # sort by (core, g, q, window)
    key = ((core * NG + g) * NQ + q) * WG + wi
    nbins_pc = NG * NQ * WG
    order = np.argsort(key, kind='stable')
    o_tab = tab_row[order]
    o_dst = dst[order]
    o_key = key[order]
    o_core = core[order]

    cnt_all = np.bincount(key, minlength=NCORES * nbins_pc)
    bin_start = np.concatenate([[0], np.cumsum(cnt_all)])[:-1]
    rank = np.arange(E, dtype=np.int64) - bin_start[o_key]   # within (c,g,q,w)

    # every window must have at least one real edge on SOME core, so the
    # shared schedule has a stop-flag anchor for each PSUM region
    wcnt = cnt_all.reshape(NCORES, NG, NQ, WG).sum(axis=(0, 2)).reshape(-1)
    assert wcnt[:NW].min() >= 1

    # shared schedule: per (g,q,w) segment length = max count over cores
    # (edge granularity); each (g,q) bin padded to a 128 multiple once.
    maxcnt = cnt_all.reshape(NCORES, NG, NQ, WG).max(axis=0)  # [NG, NQ, WG]
    seg_end = np.cumsum(maxcnt, axis=2)
    wstart = seg_end - maxcnt                                 # [NG, NQ, WG]
    binlen = ((seg_end[:, :, -1] + WIN - 1) // WIN) * WIN     # [NG, NQ]
    offs_gq = np.concatenate([[0], np.cumsum(binlen.reshape(-1))])  # per (g,q)
    TOT = int(offs_gq[-1])
    NBLK = TOT // WIN

    gqw_in_core = o_key % nbins_pc
    gq_in_core = gqw_in_core // WG
    pos = (offs_gq[gq_in_core] + wstart.reshape(-1)[gqw_in_core] + rank)
    blkid = pos // WIN                                        # global block

    # per-block overlapped windows from the shared segment layout
    wmin_blk = np.zeros(NBLK, np.int64)
    blk_wins = [[] for _ in range(NBLK)]
    for gg in range(NG):
        for qq in range(NQ):
            bin_i = gg * NQ + qq
            b0 = int(offs_gq[bin_i]) // WIN
            nb = int(binlen[gg, qq]) // WIN
            for wii in range(WG):
                w = gg * WG + wii
                if w >= NW or maxcnt[gg, qq, wii] == 0:
                    continue
                lo = int(wstart[gg, qq, wii])
                hi = int(seg_end[gg, qq, wii])
                for b in range(b0 + lo // WIN, b0 + (hi - 1) // WIN + 1):
                    blk_wins[b].append(w)
            for b in range(b0, b0 + nb):
                assert len(blk_wins[b]) <= 2, "block spans >2 windows"
                wmin_blk[b] = blk_wins[b][0] if blk_wins[b] else 0

    rel = (o_dst - (o_core * SH + wmin_blk[blkid] * WIN)).astype(np.float32)
    assert rel.min() >= 0 and rel.max() <= 255.0

    per_core = []
    for c in range(NCORES):
        m = o_core == c
        idxq = np.zeros(TOT, np.int16)
        rels = np.full(TOT, -1.0, np.float32)
        p_c = pos[m]
        idxq[p_c] = o_tab[m].astype(np.int16)
        rels[p_c] = rel[m]
        idx16 = np.tile(np.ascontiguousarray(idxq.reshape(TOT // 16, 16).T), (8, 1))
        relm = np.ascontiguousarray(rels.reshape(NBLK, WIN).T)   # [128, NBLK]
        dinv_own = np.ascontiguousarray(
            dinv_np[c * SH:(c + 1) * SH].reshape(NW, WIN).T)     # [128, NW]
        per_core.append(dict(idx16=idx16, reldst=relm, dinv=dinv_own))

    # build calls: per (g, q): gather binlen[g,q] rows; per block the
    # matmul list [(Bcol, w, bank)] for each overlapped window
    calls = []          # (g, q, off_idx, nidx, [(Bcol, w, bank), ...])
    last_block_of_win = {}
    for gg in range(NG):
        for qq in range(NQ):
            nlen = int(binlen[gg, qq])
            if nlen == 0:
                continue
            bin_i = gg * NQ + qq
            off_idx = int(offs_gq[bin_i])
            b0 = off_idx // WIN
            mms = []
            for b in range(b0, b0 + nlen // WIN):
                for w in blk_wins[b]:
                    mms.append((b, w, w - blk_wins[b][0]))
                    last_block_of_win[w] = (b, w)
            calls.append((gg, qq, off_idx, nlen, mms))
    sched = dict(calls=calls, last_block=last_block_of_win,
                 NBLK=NBLK, TOT=TOT)
    return sched, per_core, dinv_np


def _build(sched, fast_epilogue=None, self_mode="sel"):
    from concourse import bass, bacc, tile, mybir

    nc = bacc.Bacc("TRN2", target_bir_lowering=False, debug=False,
                   enable_asserts=True, num_devices=NCORES)

    xt_d = nc.dram_tensor("xt_perm", [KIN, NP], mybir.dt.bfloat16, kind="ExternalInput")
    w_d = nc.dram_tensor("w_bf", [KIN, H], mybir.dt.bfloat16, kind="ExternalInput")
    b_d = nc.dram_tensor("b_vec", [H], mybir.dt.float32, kind="ExternalInput")
    a_d = nc.dram_tensor("a_vec", [H], mybir.dt.float32, kind="ExternalInput")
    dinv_d = nc.dram_tensor("dinv_own", [128, NW], mybir.dt.float32, kind="ExternalInput")
    idx_d = nc.dram_tensor("idx16", [128, sched["TOT"] // 16], mybir.dt.int16, kind="ExternalInput")
    rel_d = nc.dram_tensor("reldst", [128, sched["NBLK"]], mybir.dt.float32, kind="ExternalInput")

    # output in partition-major layout: out[d, w*H + f] = result[w*128 + d, f]
    out_d = nc.dram_tensor("out_pm", [128, NW * H], mybir.dt.bfloat16, kind="ExternalOutput")

    hs_tab = [nc.dram_tensor(f"hs_tab{k}", [QTAB, H], mybir.dt.bfloat16) for k in range(NQ)]

    calls = sched["calls"]
    last_block = sched["last_block"]
    first_block = {}
    for (_gg, _qq, _oi, _ni, _mms) in calls:
        for (b, w, _bank) in _mms:
            first_block.setdefault(w, (b, w))
    max_call_blk = max(cb[3] // WIN for cb in calls)
    nblk_of_group = [sum(cb[3] // WIN for cb in calls if cb[0] == gg) for gg in range(NG)]
    first_col_of_group = [min([cb[2] // WIN for cb in calls if cb[0] == gg] or [0])
                          for gg in range(NG)]

    with tile.TileContext(nc) as tc:
        with tc.tile_pool(name="consts", bufs=1) as cp, tc.tile_pool(name="sb", bufs=3) as sb:
            # ---------------- constants ----------------
            iota_bank = []
            for j in range(2):
                it = cp.tile([128, 128], mybir.dt.int32, tag=f"it{j}")
                nc.gpsimd.iota(it[:], pattern=[[1, 128]], base=j * 128,
                               channel_multiplier=0)
                ib = cp.tile([128, 128], mybir.dt.bfloat16, tag=f"ib{j}")
                nc.vector.tensor_copy(ib[:], it[:])
                iota_bank.append(ib)

            # selection matrices for self-loop rows: sel[r][c][p, d] = 1 iff
            # d == 4p - 128r + c  (window w = 4*g_own + r, chunk c)
            selmat = []
            for r in range(4):
                row = []
                for c in range(4):
                    cv = cp.tile([128, 1], mybir.dt.int32, tag=f"cv{r}{c}")
                    nc.gpsimd.iota(cv[:], pattern=[[0, 1]], base=c - 128 * r,
                                   channel_multiplier=4)
                    cvf = cp.tile([128, 1], mybir.dt.float32, tag=f"cvf{r}{c}")
                    nc.vector.tensor_copy(cvf[:], cv[:])
                    sm = cp.tile([128, 128], mybir.dt.bfloat16, tag=f"sm{r}{c}")
                    nc.vector.tensor_scalar(
                        out=sm[:], in0=iota_bank[0][:],
                        scalar1=cvf[:], scalar2=None,
                        op0=mybir.AluOpType.is_equal)
                    row.append(sm)
                selmat.append(row)

            w0 = cp.tile([128, H], mybir.dt.bfloat16)
            w1 = cp.tile([128, H], mybir.dt.bfloat16)
            nc.sync.dma_start(w0[:], w_d[0:128, :])
            nc.sync.dma_start(w1[:], w_d[128:256, :])

            dinv_sb = cp.tile([128, NW], mybir.dt.float32)
            nc.sync.dma_start(dinv_sb[:], dinv_d[:, :])

            ones1 = cp.tile([1, H], mybir.dt.float32)
            nc.vector.memset(ones1[:], 1.0)
            bvec = cp.tile([1, H], mybir.dt.float32)
            nc.sync.dma_start(bvec[:], b_d[None, :])
            avec = cp.tile([1, H], mybir.dt.float32)
            nc.sync.dma_start(avec[:], a_d[None, :])

            b128 = cp.tile([128, H], mybir.dt.float32)
            a128 = cp.tile([128, H], mybir.dt.float32)

            hs_own = cp.tile([128, NOWN, GRP], mybir.dt.bfloat16)  # 25KB/part

            with tc.tile_pool(name="psum1", bufs=1, space="PSUM") as pp1:
                if not fast_epilogue:
                    bc_ps = pp1.tile([128, H], mybir.dt.float32, space="PSUM", tag="bc", bufs=1)
                    nc.tensor.matmul(out=bc_ps[:], lhsT=ones1[:], rhs=bvec[:], start=True, stop=True)
                    nc.vector.tensor_copy(b128[:], bc_ps[:])
                    ac_ps = pp1.tile([128, H], mybir.dt.float32, space="PSUM", tag="bc", bufs=1)
                    nc.tensor.matmul(out=ac_ps[:], lhsT=ones1[:], rhs=avec[:], start=True, stop=True)
                    nc.vector.tensor_copy(a128[:], ac_ps[:])

                # ---------------- phase 1: full hs table, local ----------------
                for bb in range(NGRP // XB):
                    g0 = bb * XB
                    x_t = sb.tile([128, 2, XB * GRP], mybir.dt.bfloat16, tag="x_t", bufs=3)
                    nc.sync.dma_start(
                        x_t[:],
                        xt_d[:, g0 * GRP:(g0 + XB) * GRP].rearrange(
                            "(a p) c -> p a c", p=128))
                    hb = sb.tile([128, XB, 4 * H], mybir.dt.bfloat16, tag="hb", bufs=3)
                    for gi in range(XB):
                        g = g0 + gi
                        ps = pp1.tile([128, 4 * H], mybir.dt.float32, space="PSUM",
                                      tag="h_ps", bufs=3)
                        for cc in range(4):
                            for a in range(2):
                                nc.tensor.matmul(
                                    out=ps[:, cc * H:(cc + 1) * H],
                                    lhsT=x_t[:, a, (gi * 4 + cc) * 128:(gi * 4 + cc + 1) * 128],
                                    rhs=(w0 if a == 0 else w1)[:],
                                    start=(a == 0), stop=(a == 1))
                        nc.scalar.activation(hb[:, gi, :], ps[:],
                                             mybir.ActivationFunctionType.Copy)
                        if g < NOWN:
                            nc.vector.tensor_copy(hs_own[:, g, :], ps[:])
                    # write XB groups; split at quarter boundaries
                    r_lo = g0 * GRP
                    while r_lo < (g0 + XB) * GRP:
                        qk = r_lo // QTAB
                        r_hi = min((g0 + XB) * GRP, (qk + 1) * QTAB)
                        gl = (r_lo - g0 * GRP) // GRP
                        gh = (r_hi - g0 * GRP) // GRP
                        dview = hs_tab[qk][r_lo - qk * QTAB:r_hi - qk * QTAB, :]
                        if gh - gl == 1:
                            nc.scalar.dma_start(
                                dview.rearrange("(p c) k -> p (c k)", p=128, c=4),
                                hb[:, gl, :])
                        else:
                            nc.scalar.dma_start(
                                dview.rearrange("(gi p c) k -> p gi (c k)", p=128, c=4),
                                hb[:, gl:gh, :])
                        r_lo = r_hi

            # ---------------- phase 3 ----------------
            with tc.tile_pool(name="psum3", bufs=WG, space="PSUM") as pp3:
                # prefetch first idx streams
                idx_tiles = {}
                for ci in range(min(PREF, len(calls))):
                    (gg0, qq0, off_idx, nidx, _mm) = calls[ci]
                    idx_sb = sb.tile([128, max_call_blk * 8], mybir.dt.int16,
                                     tag="idx", bufs=PREF + 1)
                    nc.sync.dma_start(idx_sb[:, 0:nidx // 16],
                                      idx_d[:, off_idx // 16: (off_idx + nidx) // 16])
                    idx_tiles[ci] = idx_sb

                for gg in range(NG):
                    wlo = gg * WG
                    whi = min(wlo + WG, NW)
                    nwin = whi - wlo
                    pw = {}
                    for w in range(wlo, whi):
                        pwt = pp3.tile([128, H], mybir.dt.float32, space="PSUM",
                                       tag="pw", name=f"pw{w}", bufs=8)
                        pw[w] = pwt[:]
                        if self_mode == "sel":
                            # self-loop contribution from SBUF-resident own rows
                            r = w % 4
                            for c in range(4):
                                nc.tensor.matmul(
                                    out=pw[w], lhsT=selmat[r][c][:],
                                    rhs=hs_own[:, w // 4, c * 128:(c + 1) * 128],
                                    start=(c == 0), stop=False)

                    rd_sb = sb.tile([128, max(nblk_of_group)], mybir.dt.float32,
                                    tag="rd", bufs=3)
                    c0 = first_col_of_group[gg]
                    nc.sync.dma_start(rd_sb[:, 0:nblk_of_group[gg]],
                                      rel_d[:, c0:c0 + nblk_of_group[gg]])

                    for ci, (g_c, qq, off_idx, nidx, mms) in enumerate(calls):
                        if g_c != gg:
                            continue
                        if ci in idx_tiles:
                            idx_sb = idx_tiles.pop(ci)
                        else:
                            idx_sb = sb.tile([128, max_call_blk * 8], mybir.dt.int16,
                                             tag="idx", bufs=PREF + 1)
                            nc.sync.dma_start(idx_sb[:, 0:nidx // 16],
                                              idx_d[:, off_idx // 16: (off_idx + nidx) // 16])
                        # prefetch a later idx stream to stay ahead
                        cn = ci + PREF
                        if cn < len(calls) and cn not in idx_tiles:
                            (_g2, _q2, off2, nidx2, _m2) = calls[cn]
                            nx = sb.tile([128, max_call_blk * 8], mybir.dt.int16,
                                         tag="idx", bufs=PREF + 1)
                            nc.sync.dma_start(nx[:, 0:nidx2 // 16],
                                              idx_d[:, off2 // 16: (off2 + nidx2) // 16])
                            idx_tiles[cn] = nx
                        g_t = sb.tile([128, max_call_blk, H], mybir.dt.bfloat16,
                                      tag="g_t", bufs=7)
                        nc.gpsimd.dma_gather(
                            g_t[:, 0:nidx // 128, :], hs_tab[qq][:, :],
                            idx_sb[:, 0:nidx // 16], nidx, nidx, H,
                            single_packet=False)
                        b0 = off_idx // WIN
                        for (bcol, w, bank) in mms:
                            s_t = sb.tile([128, 128], mybir.dt.bfloat16, tag="s_t", bufs=16)
                            lc = bcol - first_col_of_group[gg]
                            nc.vector.tensor_scalar(
                                out=s_t[:], in0=iota_bank[bank][:],
                                scalar1=rd_sb[:, lc:lc + 1], scalar2=None,
                                op0=mybir.AluOpType.is_equal)
                            nc.tensor.matmul(out=pw[w], lhsT=s_t[:],
                                             rhs=g_t[:, bcol - b0, :],
                                             start=(self_mode != "sel"
                                                    and first_block.get(w) == (bcol, w)),
                                             stop=(last_block.get(w) == (bcol, w)))

                    # epilogue, batched partition-major output DMA per group
                    o_g = sb.tile([128, WG * H], mybir.dt.bfloat16, tag="o_g", bufs=2)
                    for w in range(wlo, whi):
                        og = o_g[:, (w - wlo) * H:(w - wlo + 1) * H]
                        if fast_epilogue:
                            # b == 0, uniform alpha<=1: out = max(dinv*psum,
                            # alpha*dinv*psum)
                            al = float(fast_epilogue["alpha"])
                            u = sb.tile([128, H], mybir.dt.float32, tag="u", bufs=4)
                            nc.scalar.activation(u[:], pw[w],
                                                 mybir.ActivationFunctionType.Copy,
                                                 scale=dinv_sb[:, w:w + 1])
                            t = sb.tile([128, H], mybir.dt.float32, tag="t", bufs=4)
                            nc.vector.tensor_scalar(
                                out=t[:], in0=pw[w],
                                scalar1=dinv_sb[:, w:w + 1], scalar2=al,
                                op0=mybir.AluOpType.mult, op1=mybir.AluOpType.mult)
                            nc.vector.tensor_tensor(out=og, in0=u[:], in1=t[:],
                                                    op=mybir.AluOpType.max)
                        else:
                            u = sb.tile([128, H], mybir.dt.float32, tag="u", bufs=4)
                            nc.scalar.activation(u[:], pw[w],
                                                 mybir.ActivationFunctionType.Copy,
                                                 scale=dinv_sb[:, w:w + 1])
                            u2 = sb.tile([128, H], mybir.dt.float32, tag="u2", bufs=4)
                            nc.vector.tensor_tensor(out=u2[:], in0=u[:], in1=b128[:],
                                                    op=mybir.AluOpType.add)
                            r2 = sb.tile([128, H], mybir.dt.float32, tag="r2", bufs=3)
                            nc.scalar.activation(r2[:], u2[:],
                                                 mybir.ActivationFunctionType.Relu,
                                                 scale=-1.0)
                            m = sb.tile([128, H], mybir.dt.float32, tag="m", bufs=3)
                            nc.gpsimd.tensor_tensor(out=m[:], in0=r2[:], in1=a128[:],
                                                    op=mybir.AluOpType.mult)
                            r1 = sb.tile([128, H], mybir.dt.float32, tag="r1", bufs=3)
                            nc.scalar.activation(r1[:], u2[:],
                                                 mybir.ActivationFunctionType.Relu)
                            nc.vector.tensor_tensor(out=og,
                                                    in0=r1[:], in1=m[:],
                                                    op=mybir.AluOpType.subtract)
                    if gg == NG - 1 and nwin >= 2:
                        h1 = nwin // 2
                        nc.scalar.dma_start(out_d[:, wlo * H:(wlo + h1) * H],
                                            o_g[:, 0:h1 * H])
                        nc.scalar.dma_start(out_d[:, (wlo + h1) * H:whi * H],
                                            o_g[:, h1 * H:nwin * H])
                    else:
                        nc.scalar.dma_start(
                            out_d[:, wlo * H:whi * H],
                            o_g[:, 0:nwin * H])

    nc.compile()
    return nc


_LAST = {}


def kernel(x, edge_index, W, b, alpha):
    from concourse.bass_utils import run_bass_kernel_spmd

    x = np.asarray(x, dtype=np.float32)
    W = np.asarray(W, dtype=np.float32)
    b = np.asarray(b, dtype=np.float32)
    alpha = np.asarray(alpha, dtype=np.float32)

    sched, per_core, dinv_np = _preprocess(edge_index)
    fast = None
    if np.all(b == 0.0) and np.all(alpha == alpha.flat[0]) and 0.0 <= alpha.flat[0] <= 1.0:
        fast = {"alpha": float(alpha.flat[0])}
    nc = _build(sched, fast_epilogue=fast)
    _LAST["nc"] = nc
    _LAST["sched"] = sched

    # x' = dinv * x, padded; per-core: table slot t holds node (t + c*SH) % NP;
    # K-major columns permuted so col (g*4+cc)*128 + p holds slot g*512+p*4+cc.
    x_pad = np.zeros((NP, KIN), np.float32)
    x_pad[:N] = dinv_np[:N, None] * x
    slot_perm = np.arange(NP).reshape(NGRP, 128, 4).transpose(0, 2, 1).reshape(-1)
    # slot_perm[(g*4+cc)*128 + p] = g*512 + p*4 + cc

    w_bf = W.astype(bf16)

    in_maps = []
    for c in range(NCORES):
        node_of_col = (slot_perm + c * SH) % NP
        xt_c = np.ascontiguousarray(x_pad[node_of_col].T.astype(bf16))  # [256, NP]
        in_maps.append({
            "xt_perm": xt_c,
            "w_bf": w_bf, "b_vec": b, "a_vec": alpha,
            "dinv_own": per_core[c]["dinv"],
            "idx16": per_core[c]["idx16"],
            "reldst": per_core[c]["reldst"],
        })

    res = run_bass_kernel_spmd(nc, in_maps, core_ids=list(range(NCORES)))
    # out_pm[d, w*H+f] -> rows w*128+d
    outs = []
    for c in range(NCORES):
        o = res.results[c]["out_pm"].astype(np.float32).reshape(128, NW, H).transpose(1, 0, 2)
        outs.append(o.reshape(SH, H))
    out = np.concatenate(outs, axis=0)
    return np.ascontiguousarray(out[:N])
